# revision 9
# baseline (speedup 1.0000x reference)
"""MinGRU Trainium2 kernel.

Problem: x (8, 4096, 1024) fp32; Wz, Wh (1024, 1024); bz, bh (1024,).
    k = x @ Wz.T + bz ; z = sigmoid(k)
    p = x @ Wh.T + bh ; g = where(p >= 0, p + 0.5, sigmoid(p))
    h_t = (1 - z_t) * h_{t-1} + z_t * g_t   (h_0 = 0.5)
The reference computes this recurrence with a log-space parallel scan; here it
is computed directly in linear space (mathematically identical), using the DVE
TensorTensorScanArith instruction along the free axis.

Sharding: data-parallel over batch, one batch element per NeuronCore (8 cores).

Per-core layout: everything lives transposed, H on partitions, S on the free
axis.  The two GEMMs run in fp8 e4m3 with perf_mode=DoubleRow (2 weights per
PE cell, K=256 per matmul -> half the matmul count of full-rate fp32).  Inputs
are quantized host-side with power-of-two scales (x*16, W*1024); the exact
descale 2^-14 is folded into the ScalarE activation `scale` argument.

Work is chunked in 1024-wide sequence units (PSUM tiles span 2 banks) to
amortize the per-instruction overheads (~352 cycles per ACT, ~200 per DVE op)
and halve the semaphore traffic.  Engine assignment balances the elementwise
work across ScalarE/GpSimd/DVE so the fp8 PE stream (~130 us) stays the
critical resource:
    ScalarE: z = sigmoid(kp), sp = sigmoid(pp), rp = relu(pp)   (bias fused)
    GpSimd:  g1 = min(sp, 0.5) ; g = g1 + rp     (STT is DVE-only on v3)
    DVE:     a = 1 - z ; b = z*g ; h = scan(a, b)
The DVE triple (a, b, scan) is issued one (unit, m) slot late: engine queues
are strict FIFO, so a scan waiting at the queue head for the GpSimd ops
would stall the whole DVE — with the one-slot delay its inputs are always
ready.  All gate tensors are bf16 (uniform-dtype DVE ops run at 2x; the scan
carries fp32 state internally).  Measured end-to-end rel-err ~1.3e-2 against
the fp32 reference, within the 2e-2 budget.
"""

import os
import sys

import numpy as np

for _p in ("/opt/trn_rl_repo", "/root/.axon_site/_ro/trn_rl_repo"):
    if os.path.isdir(_p) and _p not in sys.path:
        sys.path.insert(0, _p)

import ml_dtypes  # noqa: E402

import concourse.bass as bass  # noqa: E402
import concourse.mybir as mybir  # noqa: E402
import concourse.tile as tile  # noqa: E402
from concourse import bacc  # noqa: E402
from concourse.bass_utils import run_bass_kernel_spmd  # noqa: E402

F32 = mybir.dt.float32
F32R = mybir.dt.float32r
BF16 = mybir.dt.bfloat16
F8 = mybir.dt.float8e4  # TRN e4m3 (bias 8, max +-240) == ml_dtypes.float8_e4m3
NP_F8 = ml_dtypes.float8_e4m3
NP_BF16 = ml_dtypes.bfloat16
N_CORES = 8
B, S, D, H = 8, 4096, 1024, 1024
NK = D // 128  # 8 k-tiles of 128
NKP = NK // 2  # 4 DoubleRow k-pairs
NM = H // 128

# power-of-two quantization scales; descale folded into the activations
SX = 16.0
SW = 1024.0
DESCALE = 1.0 / (SX * SW)

_cache: dict = {}


def build_nc(seq_len: int = S, n_cores: int = N_CORES):
    """Build and compile the per-core Bass module (SPMD, identical program)."""
    tsp = min(1024, seq_len)  # strip width (2 PSUM banks of fp32 at 1024)
    nst = seq_len // tsp
    nc = bacc.Bacc(
        "TRN2", target_bir_lowering=False, debug=False, num_devices=n_cores
    )

    # x packed host-side as [p, strip, ktile, t] so one DMA fetches a strip
    xp_d = nc.dram_tensor("xp8", [128, nst, NK, tsp], F8, kind="ExternalInput")
    # weights packed as [p, ktile, m] (wz8[p, kt, m] = Wz[m, kt*128+p] * SW)
    wz_d = nc.dram_tensor("wz8", [128, NK, H], F8, kind="ExternalInput")
    wh_d = nc.dram_tensor("wh8", [128, NK, H], F8, kind="ExternalInput")
    bz_d = nc.dram_tensor("bz", [H], F32, kind="ExternalInput")
    bh_d = nc.dram_tensor("bh", [H], F32, kind="ExternalInput")
    hT_d = nc.dram_tensor("hT", [H, seq_len], BF16, kind="ExternalOutput")

    AF = mybir.ActivationFunctionType
    OP = mybir.AluOpType
    DR = mybir.MatmulPerfMode.DoubleRow

    with tile.TileContext(nc) as tc:
        with (
            tc.tile_pool(name="singles", bufs=1) as singles,
            tc.tile_pool(name="xs", bufs=3) as xpool,
            tc.tile_pool(name="work", bufs=3) as work,
            tc.tile_pool(name="hbuf", bufs=2) as hpool,
            tc.tile_pool(name="psum", bufs=2, space="PSUM") as psum,
        ):
            # PE warm-up: the HAM clock gate holds the PE at 1.2 GHz until it
            # has been busy ~3.4 us.  The PE sits idle anyway while the first
            # DMAs land, so burn that time on dummy matmuls over a zeroed
            # tile — the first real matmuls then run at 2.4 GHz.
            warm = singles.tile([128, 256], F32, tag="warm")
            nc.gpsimd.memset(warm[:], 0.0)
            wps = psum.tile([128, tsp], F32, tag="kp")
            for i in range(10):
                nc.tensor.matmul(
                    wps[:, :256], lhsT=warm[:, :128].bitcast(F32R),
                    rhs=warm[:].bitcast(F32R),
                    start=(i == 0), stop=(i == 9),
                )
            # Biases first: tiny but they gate every activation.
            bz_sb = singles.tile([128, NM], F32, tag="bz")
            nc.sync.dma_start(out=bz_sb, in_=bz_d.ap().rearrange("(m p) -> p m", p=128))
            bh_sb = singles.tile([128, NM], F32, tag="bh")
            nc.sync.dma_start(out=bh_sb, in_=bh_d.ap().rearrange("(m p) -> p m", p=128))
            # First strip of x + the m<4 half of the weights, interleaved per
            # k-pair so matmul (s=0, m=0, j=0) unblocks after ~3 transfers.
            xs0 = xpool.tile([128, NK, tsp], F8, tag="xs")
            wz_sb = singles.tile([128, NK, H], F8, tag="wz")
            wh_sb = singles.tile([128, NK, H], F8, tag="wh")
            for j in range(NKP):
                ksl = slice(2 * j, 2 * j + 2)
                nc.sync.dma_start(out=xs0[:, ksl, :], in_=xp_d.ap()[:, 0, ksl, :])
                nc.sync.dma_start(out=wz_sb[:, ksl, :H // 2],
                                  in_=wz_d.ap()[:, ksl, :H // 2])
                nc.sync.dma_start(out=wh_sb[:, ksl, :H // 2],
                                  in_=wh_d.ap()[:, ksl, :H // 2])
            for j in range(NKP):
                ksl = slice(2 * j, 2 * j + 2)
                nc.sync.dma_start(out=wz_sb[:, ksl, H // 2:],
                                  in_=wz_d.ap()[:, ksl, H // 2:])
                nc.sync.dma_start(out=wh_sb[:, ksl, H // 2:],
                                  in_=wh_d.ap()[:, ksl, H // 2:])
            # Sequence units: full strips of `tsp`, with the final strip split
            # in half so the end-of-kernel pipeline drain runs on narrower
            # tiles.
            units = [(s, 0, tsp) for s in range(nst - 1)]
            units += [(nst - 1, 0, tsp // 2), (nst - 1, tsp // 2, tsp // 2)]
            h_prev: list = [None] * NM
            pending: list = []

            def gate_front(m, kp, pp, tw, ts_sl):
                """ScalarE + GpSimd gate math for one (unit, m) slot."""
                z = work.tile([128, tsp], BF16, tag="z")
                nc.scalar.activation(
                    out=z[:, :tw], in_=kp[:, :tw], func=AF.Sigmoid,
                    bias=bz_sb[:, m:m + 1], scale=DESCALE,
                )
                sp = work.tile([128, tsp], BF16, tag="sp")
                nc.scalar.activation(
                    out=sp[:, :tw], in_=pp[:, :tw], func=AF.Sigmoid,
                    bias=bh_sb[:, m:m + 1], scale=DESCALE,
                )
                rp = work.tile([128, tsp], BF16, tag="rp")
                nc.scalar.activation(
                    out=rp[:, :tw], in_=pp[:, :tw], func=AF.Relu,
                    bias=bh_sb[:, m:m + 1], scale=DESCALE,
                )
                # g = min(sigmoid(p+bh), 0.5) + relu(p+bh), split in two
                # GpSimd-legal ops (scalar_tensor_tensor is DVE-only)
                g1 = work.tile([128, tsp], BF16, tag="g1")
                nc.gpsimd.tensor_scalar(
                    out=g1[:, :tw], in0=sp[:, :tw], scalar1=0.5, scalar2=None,
                    op0=OP.min,
                )
                g = work.tile([128, tsp], BF16, tag="g")
                nc.gpsimd.tensor_tensor(
                    out=g[:, :tw], in0=g1[:, :tw], in1=rp[:, :tw], op=OP.add
                )
                pending.append((m, z, g, tw, ts_sl))

            def gate_back():
                """DVE a + b + scan + store, one slot behind gate_front."""
                m, z, g, tw, ts_sl = pending.pop(0)
                a = work.tile([128, tsp], BF16, tag="a")
                nc.vector.tensor_scalar(
                    out=a[:, :tw], in0=z[:, :tw], scalar1=-1.0, scalar2=1.0,
                    op0=OP.mult, op1=OP.add,
                )
                b = work.tile([128, tsp], BF16, tag="b")
                nc.vector.tensor_tensor(
                    out=b[:, :tw], in0=z[:, :tw], in1=g[:, :tw], op=OP.mult
                )
                # h_t = a_t * h_{t-1} + b_t along the free axis
                h = hpool.tile([128, tsp], BF16, tag=f"h{m}")
                if h_prev[m] is None:
                    init = 0.5
                else:
                    pt, pw = h_prev[m]
                    init = pt[:, pw - 1:pw]
                nc.vector.tensor_tensor_scan(
                    out=h[:, :tw], data0=a[:, :tw], data1=b[:, :tw],
                    initial=init, op0=OP.mult, op1=OP.add,
                )
                h_prev[m] = (h, tw)
                nc.sync.dma_start(out=hT_d.ap()[m * 128:(m + 1) * 128, ts_sl],
                                  in_=h[:, :tw])

            for u, (sidx, off, tw) in enumerate(units):
                ts0 = sidx * tsp + off
                ts_sl = slice(ts0, ts0 + tw)
                if sidx == 0:
                    xs = xs0
                elif off == 0:
                    xs = xpool.tile([128, NK, tsp], F8, tag="xs")
                    nc.sync.dma_start(out=xs, in_=xp_d.ap()[:, sidx, :, :])
                # (tail sub-units reuse the strip tile loaded at off==0)
                blocks = [(off + i, min(512, tw - i)) for i in range(0, tw, 512)]
                for m in range(NM):
                    m_sl = slice(m * 128, (m + 1) * 128)
                    kp = psum.tile([128, tsp], F32, tag="kp")
                    pp = psum.tile([128, tsp], F32, tag="pp")
                    for wsb, out_ps in ((wz_sb, kp), (wh_sb, pp)):
                        for j in range(NKP):
                            ksl = slice(2 * j, 2 * j + 2)
                            for bo, bw in blocks:
                                nc.tensor.matmul(
                                    out_ps[:, bo - off:bo - off + bw],
                                    lhsT=wsb[:, ksl, m_sl],
                                    rhs=xs[:, ksl, bo:bo + bw],
                                    start=(j == 0),
                                    stop=(j == NKP - 1),
                                    perf_mode=DR,
                                )
                    gate_front(m, kp, pp, tw, ts_sl)
                    if len(pending) > 1:
                        gate_back()
            while pending:
                gate_back()

    nc.compile()
    return nc


def quantize_pack_x(x_b: np.ndarray, seq_len: int = S) -> np.ndarray:
    """x_b (seq, D) fp32 -> packed [128, nst, NK, tsp] fp8 (scaled by SX)."""
    tsp = min(1024, seq_len)
    nst = seq_len // tsp
    x8 = np.asarray(x_b * SX, dtype=NP_F8)
    return np.ascontiguousarray(
        x8.reshape(nst, tsp, NK, 128).transpose(3, 0, 2, 1)
    )


def quantize_pack_w(W: np.ndarray) -> np.ndarray:
    """W (H, D) fp32 -> packed [128, NK, H] fp8 (scaled by SW)."""
    W8 = np.asarray(W * SW, dtype=NP_F8)
    # w8[p, kt, m] = W[m, kt*128+p] * SW
    return np.ascontiguousarray(W8.T.reshape(NK, 128, H).transpose(1, 0, 2))


def make_in_maps(x, Wz, bz, Wh, bh, seq_len: int = S):
    wz8 = quantize_pack_w(np.asarray(Wz, np.float32))
    wh8 = quantize_pack_w(np.asarray(Wh, np.float32))
    bz = np.ascontiguousarray(bz, dtype=np.float32)
    bh = np.ascontiguousarray(bh, dtype=np.float32)
    return [
        {
            "xp8": quantize_pack_x(np.asarray(x[b], np.float32), seq_len),
            "wz8": wz8,
            "wh8": wh8,
            "bz": bz,
            "bh": bh,
        }
        for b in range(x.shape[0])
    ]


def kernel(x, Wz, bz, Wh, bh):
    x = np.ascontiguousarray(x, dtype=np.float32)
    key = "nc"
    if key not in _cache:
        _cache[key] = build_nc()
    nc = _cache[key]

    in_maps = make_in_maps(x, Wz, bz, Wh, bh)
    res = run_bass_kernel_spmd(nc, in_maps, list(range(N_CORES)))
    out = np.empty((B, S, H), np.float32)
    for b in range(N_CORES):
        out[b] = res.results[b]["hT"].astype(np.float32).T
    return out


# revision 11
# speedup vs baseline: 2.5010x; 2.5010x over previous
"""MinGRU Trainium2 kernel.

Problem: x (8, 4096, 1024) fp32; Wz, Wh (1024, 1024); bz, bh (1024,).
    k = x @ Wz.T + bz ; z = sigmoid(k)
    p = x @ Wh.T + bh ; g = where(p >= 0, p + 0.5, sigmoid(p))
    h_t = (1 - z_t) * h_{t-1} + z_t * g_t   (h_0 = 0.5)
The reference computes this recurrence with a log-space parallel scan; here it
is computed directly in linear space (mathematically identical), using the DVE
TensorTensorScanArith instruction along the free axis.

Sharding: data-parallel over batch, one batch element per NeuronCore (8 cores).

Per-core layout: everything lives transposed, H on partitions, S on the free
axis.  The two GEMMs run in fp8 e4m3 with perf_mode=DoubleRow (2 weights per
PE cell, K=256 per matmul -> half the matmul count of full-rate fp32).  Inputs
are quantized host-side with power-of-two scales (x*16, W*1024); the exact
descale 2^-14 is folded into the ScalarE activation `scale` argument.

Work is chunked in 1024-wide sequence units (PSUM tiles span 2 banks) to
amortize the per-instruction overheads (~352 cycles per ACT, ~200 per DVE op)
and halve the semaphore traffic.  Engine assignment balances the elementwise
work across ScalarE/GpSimd/DVE so the fp8 PE stream (~130 us) stays the
critical resource:
    ScalarE: z = sigmoid(kp), sp = sigmoid(pp), rp = relu(pp)   (bias fused)
    DVE:     g = min(sp,.5)+rp (STT is DVE-only on v3) ; h = scan(a, b)
    GpSimd:  a = 1 - z ; b = z*g
The scan is issued one (unit, m) slot late: engine queues are strict FIFO,
so a scan waiting at the queue head for the GpSimd b would stall the whole
DVE — with the one-slot delay its inputs are always ready.  Gate tensors are
fp32 (bf16 ALU ops measured 3-10x SLOWER on DVE/GpSimd); only the stored h
is bf16 (the scan's serial rate is dtype-neutral and it halves the output
DMA).  Measured end-to-end rel-err ~1.3e-2 against the fp32 reference,
within the 2e-2 budget.
"""

import os
import sys

import numpy as np

for _p in ("/opt/trn_rl_repo", "/root/.axon_site/_ro/trn_rl_repo"):
    if os.path.isdir(_p) and _p not in sys.path:
        sys.path.insert(0, _p)

import ml_dtypes  # noqa: E402

import concourse.bass as bass  # noqa: E402
import concourse.mybir as mybir  # noqa: E402
import concourse.tile as tile  # noqa: E402
from concourse import bacc  # noqa: E402
from concourse.bass_utils import run_bass_kernel_spmd  # noqa: E402

F32 = mybir.dt.float32
F32R = mybir.dt.float32r
BF16 = mybir.dt.bfloat16
F8 = mybir.dt.float8e4  # TRN e4m3 (bias 8, max +-240) == ml_dtypes.float8_e4m3
NP_F8 = ml_dtypes.float8_e4m3
NP_BF16 = ml_dtypes.bfloat16
N_CORES = 8
B, S, D, H = 8, 4096, 1024, 1024
NK = D // 128  # 8 k-tiles of 128
NKP = NK // 2  # 4 DoubleRow k-pairs
NM = H // 128

# power-of-two quantization scales; descale folded into the activations
SX = 16.0
SW = 1024.0
DESCALE = 1.0 / (SX * SW)

_cache: dict = {}


def build_nc(seq_len: int = S, n_cores: int = N_CORES):
    """Build and compile the per-core Bass module (SPMD, identical program)."""
    tsp = min(1024, seq_len)  # strip width (2 PSUM banks of fp32 at 1024)
    nst = seq_len // tsp
    nc = bacc.Bacc(
        "TRN2", target_bir_lowering=False, debug=False, num_devices=n_cores
    )

    # x packed host-side as [p, strip, ktile, t] so one DMA fetches a strip
    xp_d = nc.dram_tensor("xp8", [128, nst, NK, tsp], F8, kind="ExternalInput")
    # weights packed as [p, ktile, m] (wz8[p, kt, m] = Wz[m, kt*128+p] * SW)
    wz_d = nc.dram_tensor("wz8", [128, NK, H], F8, kind="ExternalInput")
    wh_d = nc.dram_tensor("wh8", [128, NK, H], F8, kind="ExternalInput")
    bz_d = nc.dram_tensor("bz", [H], F32, kind="ExternalInput")
    bh_d = nc.dram_tensor("bh", [H], F32, kind="ExternalInput")
    hT_d = nc.dram_tensor("hT", [H, seq_len], BF16, kind="ExternalOutput")

    AF = mybir.ActivationFunctionType
    OP = mybir.AluOpType
    DR = mybir.MatmulPerfMode.DoubleRow

    with tile.TileContext(nc) as tc:
        with (
            tc.tile_pool(name="singles", bufs=1) as singles,
            tc.tile_pool(name="xs", bufs=3) as xpool,
            tc.tile_pool(name="work", bufs=3) as work,
            tc.tile_pool(name="hbuf", bufs=2) as hpool,
            tc.tile_pool(name="psum", bufs=2, space="PSUM") as psum,
        ):
            # PE warm-up: the HAM clock gate holds the PE at 1.2 GHz until it
            # has been busy ~3.4 us.  The PE sits idle anyway while the first
            # DMAs land, so burn that time on dummy matmuls over a zeroed
            # tile — the first real matmuls then run at 2.4 GHz.
            warm = singles.tile([128, 256], F32, tag="warm")
            nc.gpsimd.memset(warm[:], 0.0)
            wps = psum.tile([128, tsp], F32, tag="kp")
            for i in range(10):
                nc.tensor.matmul(
                    wps[:, :256], lhsT=warm[:, :128].bitcast(F32R),
                    rhs=warm[:].bitcast(F32R),
                    start=(i == 0), stop=(i == 9),
                )
            # Biases first: tiny but they gate every activation.
            bz_sb = singles.tile([128, NM], F32, tag="bz")
            nc.sync.dma_start(out=bz_sb, in_=bz_d.ap().rearrange("(m p) -> p m", p=128))
            bh_sb = singles.tile([128, NM], F32, tag="bh")
            nc.sync.dma_start(out=bh_sb, in_=bh_d.ap().rearrange("(m p) -> p m", p=128))
            # First strip of x + the m<4 half of the weights, interleaved per
            # k-pair so matmul (s=0, m=0, j=0) unblocks after ~3 transfers.
            xs0 = xpool.tile([128, NK, tsp], F8, tag="xs")
            wz_sb = singles.tile([128, NK, H], F8, tag="wz")
            wh_sb = singles.tile([128, NK, H], F8, tag="wh")
            for j in range(NKP):
                ksl = slice(2 * j, 2 * j + 2)
                nc.sync.dma_start(out=xs0[:, ksl, :], in_=xp_d.ap()[:, 0, ksl, :])
                nc.sync.dma_start(out=wz_sb[:, ksl, :H // 2],
                                  in_=wz_d.ap()[:, ksl, :H // 2])
                nc.sync.dma_start(out=wh_sb[:, ksl, :H // 2],
                                  in_=wh_d.ap()[:, ksl, :H // 2])
            for j in range(NKP):
                ksl = slice(2 * j, 2 * j + 2)
                nc.sync.dma_start(out=wz_sb[:, ksl, H // 2:],
                                  in_=wz_d.ap()[:, ksl, H // 2:])
                nc.sync.dma_start(out=wh_sb[:, ksl, H // 2:],
                                  in_=wh_d.ap()[:, ksl, H // 2:])
            # Sequence units: full strips of `tsp`, with the final strip split
            # in half so the end-of-kernel pipeline drain runs on narrower
            # tiles.
            units = [(s, 0, tsp) for s in range(nst - 1)]
            units += [(nst - 1, 0, tsp // 2), (nst - 1, tsp // 2, tsp // 2)]
            h_prev: list = [None] * NM
            pending: list = []

            def gate_front(m, kp, pp, tw, ts_sl):
                """ScalarE + DVE-g + GpSimd gate math for one (unit, m) slot."""
                z = work.tile([128, tsp], F32, tag="z")
                nc.scalar.activation(
                    out=z[:, :tw], in_=kp[:, :tw], func=AF.Sigmoid,
                    bias=bz_sb[:, m:m + 1], scale=DESCALE,
                )
                sp = work.tile([128, tsp], F32, tag="sp")
                nc.scalar.activation(
                    out=sp[:, :tw], in_=pp[:, :tw], func=AF.Sigmoid,
                    bias=bh_sb[:, m:m + 1], scale=DESCALE,
                )
                rp = work.tile([128, tsp], F32, tag="rp")
                nc.scalar.activation(
                    out=rp[:, :tw], in_=pp[:, :tw], func=AF.Relu,
                    bias=bh_sb[:, m:m + 1], scale=DESCALE,
                )
                # g = min(sigmoid(p+bh), 0.5) + relu(p+bh)
                g = work.tile([128, tsp], F32, tag="g")
                nc.vector.scalar_tensor_tensor(
                    out=g[:, :tw], in0=sp[:, :tw], scalar=0.5, in1=rp[:, :tw],
                    op0=OP.min, op1=OP.add,
                )
                # a = 1 - z
                a = work.tile([128, tsp], F32, tag="a")
                nc.gpsimd.tensor_scalar(
                    out=a[:, :tw], in0=z[:, :tw], scalar1=-1.0, scalar2=1.0,
                    op0=OP.mult, op1=OP.add,
                )
                # b = z * g
                b = work.tile([128, tsp], F32, tag="b")
                nc.gpsimd.tensor_tensor(
                    out=b[:, :tw], in0=z[:, :tw], in1=g[:, :tw], op=OP.mult
                )
                pending.append((m, a, b, tw, ts_sl))

            def gate_back():
                """DVE scan + store, one slot behind gate_front."""
                m, a, b, tw, ts_sl = pending.pop(0)
                # h_t = a_t * h_{t-1} + b_t along the free axis
                h = hpool.tile([128, tsp], BF16, tag=f"h{m}")
                if h_prev[m] is None:
                    init = 0.5
                else:
                    pt, pw = h_prev[m]
                    init = pt[:, pw - 1:pw]
                nc.vector.tensor_tensor_scan(
                    out=h[:, :tw], data0=a[:, :tw], data1=b[:, :tw],
                    initial=init, op0=OP.mult, op1=OP.add,
                )
                h_prev[m] = (h, tw)
                nc.sync.dma_start(out=hT_d.ap()[m * 128:(m + 1) * 128, ts_sl],
                                  in_=h[:, :tw])

            for u, (sidx, off, tw) in enumerate(units):
                ts0 = sidx * tsp + off
                ts_sl = slice(ts0, ts0 + tw)
                if sidx == 0:
                    xs = xs0
                elif off == 0:
                    xs = xpool.tile([128, NK, tsp], F8, tag="xs")
                    nc.sync.dma_start(out=xs, in_=xp_d.ap()[:, sidx, :, :])
                # (tail sub-units reuse the strip tile loaded at off==0)
                blocks = [(off + i, min(512, tw - i)) for i in range(0, tw, 512)]
                for m in range(NM):
                    m_sl = slice(m * 128, (m + 1) * 128)
                    kp = psum.tile([128, tsp], F32, tag="kp")
                    pp = psum.tile([128, tsp], F32, tag="pp")
                    for wsb, out_ps in ((wz_sb, kp), (wh_sb, pp)):
                        for j in range(NKP):
                            ksl = slice(2 * j, 2 * j + 2)
                            for bo, bw in blocks:
                                nc.tensor.matmul(
                                    out_ps[:, bo - off:bo - off + bw],
                                    lhsT=wsb[:, ksl, m_sl],
                                    rhs=xs[:, ksl, bo:bo + bw],
                                    start=(j == 0),
                                    stop=(j == NKP - 1),
                                    perf_mode=DR,
                                )
                    gate_front(m, kp, pp, tw, ts_sl)
                    if len(pending) > 1:
                        gate_back()
            while pending:
                gate_back()

    nc.compile()
    return nc


def quantize_pack_x(x_b: np.ndarray, seq_len: int = S) -> np.ndarray:
    """x_b (seq, D) fp32 -> packed [128, nst, NK, tsp] fp8 (scaled by SX)."""
    tsp = min(1024, seq_len)
    nst = seq_len // tsp
    x8 = np.asarray(x_b * SX, dtype=NP_F8)
    return np.ascontiguousarray(
        x8.reshape(nst, tsp, NK, 128).transpose(3, 0, 2, 1)
    )


def quantize_pack_w(W: np.ndarray) -> np.ndarray:
    """W (H, D) fp32 -> packed [128, NK, H] fp8 (scaled by SW)."""
    W8 = np.asarray(W * SW, dtype=NP_F8)
    # w8[p, kt, m] = W[m, kt*128+p] * SW
    return np.ascontiguousarray(W8.T.reshape(NK, 128, H).transpose(1, 0, 2))


def make_in_maps(x, Wz, bz, Wh, bh, seq_len: int = S):
    wz8 = quantize_pack_w(np.asarray(Wz, np.float32))
    wh8 = quantize_pack_w(np.asarray(Wh, np.float32))
    bz = np.ascontiguousarray(bz, dtype=np.float32)
    bh = np.ascontiguousarray(bh, dtype=np.float32)
    return [
        {
            "xp8": quantize_pack_x(np.asarray(x[b], np.float32), seq_len),
            "wz8": wz8,
            "wh8": wh8,
            "bz": bz,
            "bh": bh,
        }
        for b in range(x.shape[0])
    ]


def kernel(x, Wz, bz, Wh, bh):
    x = np.ascontiguousarray(x, dtype=np.float32)
    key = "nc"
    if key not in _cache:
        _cache[key] = build_nc()
    nc = _cache[key]

    in_maps = make_in_maps(x, Wz, bz, Wh, bh)
    res = run_bass_kernel_spmd(nc, in_maps, list(range(N_CORES)))
    out = np.empty((B, S, H), np.float32)
    for b in range(N_CORES):
        out[b] = res.results[b]["hT"].astype(np.float32).T
    return out


# revision 14
# speedup vs baseline: 2.5872x; 1.0345x over previous
"""MinGRU Trainium2 kernel.

Problem: x (8, 4096, 1024) fp32; Wz, Wh (1024, 1024); bz, bh (1024,).
    k = x @ Wz.T + bz ; z = sigmoid(k)
    p = x @ Wh.T + bh ; g = where(p >= 0, p + 0.5, sigmoid(p))
    h_t = (1 - z_t) * h_{t-1} + z_t * g_t   (h_0 = 0.5)
The reference computes this recurrence with a log-space parallel scan; here it
is computed directly in linear space (mathematically identical), using the DVE
TensorTensorScanArith instruction along the free axis.

Sharding: data-parallel over batch, one batch element per NeuronCore (8 cores).

Per-core layout: everything lives transposed, H on partitions, S on the free
axis.  The two GEMMs run in fp8 e4m3 with perf_mode=DoubleRow (2 weights per
PE cell, K=256 per matmul -> half the matmul count of full-rate fp32).  Inputs
are quantized host-side with power-of-two scales (x*16, W*1024); the exact
descale 2^-14 is folded into the ScalarE activation `scale` argument.

Work is chunked in 1024-wide sequence units (PSUM tiles span 2 banks) to
amortize the per-instruction overheads (~352 cycles per ACT, ~200 per DVE op)
and halve the semaphore traffic.  Engine assignment balances the elementwise
work across ScalarE/GpSimd/DVE so the fp8 PE stream (~130 us) stays the
critical resource:
    ScalarE: z = sigmoid(kp), sp = sigmoid(pp), rp = relu(pp)   (bias fused)
    DVE:     g = min(sp,.5)+rp (STT is DVE-only on v3) ; h = scan(a, b)
    GpSimd:  a = 1 - z ; b = z*g
The scan is issued one (unit, m) slot late AND ahead of g in program order:
engine queues are strict FIFO, so the op at the DVE head must always have
ready inputs — scan(u-1)'s inputs are a slot old, while g(u) may still be
waiting on the Scalar.  z/sp/rp/a stay fp32 (bf16 tensor_scalar on GpSimd
measured 10x slower); g/b/h are stored bf16 (halves SBUF traffic on the
GpSimd/DVE shared port; scan reads (a fp32, b bf16) at the same serial rate
as fp32).  Work-pool buffer counts are sized so no engine stalls on tile
reuse (the z tile is read by GpSimd up to ~2 slots late).  Measured
end-to-end rel-err ~1.3e-2 against the fp32 reference, within 2e-2.
"""

import os
import sys

import numpy as np

for _p in ("/opt/trn_rl_repo", "/root/.axon_site/_ro/trn_rl_repo"):
    if os.path.isdir(_p) and _p not in sys.path:
        sys.path.insert(0, _p)

import ml_dtypes  # noqa: E402

import concourse.bass as bass  # noqa: E402
import concourse.mybir as mybir  # noqa: E402
import concourse.tile as tile  # noqa: E402
from concourse import bacc  # noqa: E402
from concourse.bass_utils import run_bass_kernel_spmd  # noqa: E402

F32 = mybir.dt.float32
F32R = mybir.dt.float32r
BF16 = mybir.dt.bfloat16
F8 = mybir.dt.float8e4  # TRN e4m3 (bias 8, max +-240) == ml_dtypes.float8_e4m3
NP_F8 = ml_dtypes.float8_e4m3
NP_BF16 = ml_dtypes.bfloat16
N_CORES = 8
B, S, D, H = 8, 4096, 1024, 1024
NK = D // 128  # 8 k-tiles of 128
NKP = NK // 2  # 4 DoubleRow k-pairs
NM = H // 128

# power-of-two quantization scales; descale folded into the activations
SX = 16.0
SW = 1024.0
DESCALE = 1.0 / (SX * SW)

_cache: dict = {}


def build_nc(seq_len: int = S, n_cores: int = N_CORES):
    """Build and compile the per-core Bass module (SPMD, identical program)."""
    tsp = min(1024, seq_len)  # strip width (2 PSUM banks of fp32 at 1024)
    nst = seq_len // tsp
    nc = bacc.Bacc(
        "TRN2", target_bir_lowering=False, debug=False, num_devices=n_cores
    )

    # x packed host-side as [p, strip, ktile, t] so one DMA fetches a strip
    xp_d = nc.dram_tensor("xp8", [128, nst, NK, tsp], F8, kind="ExternalInput")
    # weights packed as [p, ktile, m] (wz8[p, kt, m] = Wz[m, kt*128+p] * SW)
    wz_d = nc.dram_tensor("wz8", [128, NK, H], F8, kind="ExternalInput")
    wh_d = nc.dram_tensor("wh8", [128, NK, H], F8, kind="ExternalInput")
    bz_d = nc.dram_tensor("bz", [H], F32, kind="ExternalInput")
    bh_d = nc.dram_tensor("bh", [H], F32, kind="ExternalInput")
    hT_d = nc.dram_tensor("hT", [H, seq_len], BF16, kind="ExternalOutput")

    AF = mybir.ActivationFunctionType
    OP = mybir.AluOpType
    DR = mybir.MatmulPerfMode.DoubleRow

    with tile.TileContext(nc) as tc:
        with (
            tc.tile_pool(name="singles", bufs=1) as singles,
            tc.tile_pool(name="xs", bufs=3) as xpool,
            tc.tile_pool(name="work", bufs=3) as work,
            tc.tile_pool(name="hbuf", bufs=2) as hpool,
            tc.tile_pool(name="psum", bufs=2, space="PSUM") as psum,
        ):
            # PE warm-up: the HAM clock gate holds the PE at 1.2 GHz until it
            # has been busy ~3.4 us.  The PE sits idle anyway while the first
            # DMAs land, so burn that time on dummy matmuls over a zeroed
            # tile — the first real matmuls then run at 2.4 GHz.
            warm = singles.tile([128, 256], F32, tag="warm")
            nc.gpsimd.memset(warm[:], 0.0)
            wps = psum.tile([128, tsp], F32, tag="kp")
            for i in range(10):
                nc.tensor.matmul(
                    wps[:, :256], lhsT=warm[:, :128].bitcast(F32R),
                    rhs=warm[:].bitcast(F32R),
                    start=(i == 0), stop=(i == 9),
                )
            # Biases first: tiny but they gate every activation.
            bz_sb = singles.tile([128, NM], F32, tag="bz")
            nc.sync.dma_start(out=bz_sb, in_=bz_d.ap().rearrange("(m p) -> p m", p=128))
            bh_sb = singles.tile([128, NM], F32, tag="bh")
            nc.sync.dma_start(out=bh_sb, in_=bh_d.ap().rearrange("(m p) -> p m", p=128))
            # First strip of x + the m<4 half of the weights, interleaved per
            # k-pair so matmul (s=0, m=0, j=0) unblocks after ~3 transfers.
            xs0 = xpool.tile([128, NK, tsp], F8, tag="xs")
            wz_sb = singles.tile([128, NK, H], F8, tag="wz")
            wh_sb = singles.tile([128, NK, H], F8, tag="wh")
            for j in range(NKP):
                ksl = slice(2 * j, 2 * j + 2)
                nc.sync.dma_start(out=xs0[:, ksl, :], in_=xp_d.ap()[:, 0, ksl, :])
                nc.sync.dma_start(out=wz_sb[:, ksl, :H // 2],
                                  in_=wz_d.ap()[:, ksl, :H // 2])
                nc.sync.dma_start(out=wh_sb[:, ksl, :H // 2],
                                  in_=wh_d.ap()[:, ksl, :H // 2])
            for j in range(NKP):
                ksl = slice(2 * j, 2 * j + 2)
                nc.sync.dma_start(out=wz_sb[:, ksl, H // 2:],
                                  in_=wz_d.ap()[:, ksl, H // 2:])
                nc.sync.dma_start(out=wh_sb[:, ksl, H // 2:],
                                  in_=wh_d.ap()[:, ksl, H // 2:])
            # Sequence units: full strips of `tsp`, with the final strip split
            # in half so the end-of-kernel pipeline drain runs on narrower
            # tiles.
            units = [(s, 0, tsp) for s in range(nst - 1)]
            units += [(nst - 1, 0, tsp // 2), (nst - 1, tsp // 2, tsp // 2)]
            h_prev: list = [None] * NM
            pending: list = []

            def gate_front(m, kp, pp, tw, ts_sl):
                """ScalarE + DVE-g + GpSimd gate math for one (unit, m) slot."""
                z = work.tile([128, tsp], F32, tag="z", bufs=6)
                nc.scalar.activation(
                    out=z[:, :tw], in_=kp[:, :tw], func=AF.Sigmoid,
                    bias=bz_sb[:, m:m + 1], scale=DESCALE,
                )
                sp = work.tile([128, tsp], F32, tag="sp", bufs=4)
                nc.scalar.activation(
                    out=sp[:, :tw], in_=pp[:, :tw], func=AF.Sigmoid,
                    bias=bh_sb[:, m:m + 1], scale=DESCALE,
                )
                rp = work.tile([128, tsp], F32, tag="rp", bufs=4)
                nc.scalar.activation(
                    out=rp[:, :tw], in_=pp[:, :tw], func=AF.Relu,
                    bias=bh_sb[:, m:m + 1], scale=DESCALE,
                )
                # g = min(sigmoid(p+bh), 0.5) + relu(p+bh)
                g = work.tile([128, tsp], BF16, tag="g", bufs=4)
                nc.vector.scalar_tensor_tensor(
                    out=g[:, :tw], in0=sp[:, :tw], scalar=0.5, in1=rp[:, :tw],
                    op0=OP.min, op1=OP.add,
                )
                # a = 1 - z
                a = work.tile([128, tsp], F32, tag="a", bufs=4)
                nc.gpsimd.tensor_scalar(
                    out=a[:, :tw], in0=z[:, :tw], scalar1=-1.0, scalar2=1.0,
                    op0=OP.mult, op1=OP.add,
                )
                # b = z * g
                b = work.tile([128, tsp], BF16, tag="b", bufs=4)
                nc.gpsimd.tensor_tensor(
                    out=b[:, :tw], in0=z[:, :tw], in1=g[:, :tw], op=OP.mult
                )
                pending.append((m, a, b, tw, ts_sl))

            def gate_back():
                """DVE scan + store, one slot behind gate_front."""
                m, a, b, tw, ts_sl = pending.pop(0)
                # h_t = a_t * h_{t-1} + b_t along the free axis
                h = hpool.tile([128, tsp], BF16, tag=f"h{m}")
                if h_prev[m] is None:
                    init = 0.5
                else:
                    pt, pw = h_prev[m]
                    init = pt[:, pw - 1:pw]
                nc.vector.tensor_tensor_scan(
                    out=h[:, :tw], data0=a[:, :tw], data1=b[:, :tw],
                    initial=init, op0=OP.mult, op1=OP.add,
                )
                h_prev[m] = (h, tw)
                nc.sync.dma_start(out=hT_d.ap()[m * 128:(m + 1) * 128, ts_sl],
                                  in_=h[:, :tw])

            for u, (sidx, off, tw) in enumerate(units):
                ts0 = sidx * tsp + off
                ts_sl = slice(ts0, ts0 + tw)
                if sidx == 0:
                    xs = xs0
                elif off == 0:
                    xs = xpool.tile([128, NK, tsp], F8, tag="xs")
                    nc.sync.dma_start(out=xs, in_=xp_d.ap()[:, sidx, :, :])
                # (tail sub-units reuse the strip tile loaded at off==0)
                blocks = [(off + i, min(512, tw - i)) for i in range(0, tw, 512)]
                for m in range(NM):
                    m_sl = slice(m * 128, (m + 1) * 128)
                    kp = psum.tile([128, tsp], F32, tag="kp")
                    pp = psum.tile([128, tsp], F32, tag="pp")
                    for wsb, out_ps in ((wz_sb, kp), (wh_sb, pp)):
                        for j in range(NKP):
                            ksl = slice(2 * j, 2 * j + 2)
                            for bo, bw in blocks:
                                nc.tensor.matmul(
                                    out_ps[:, bo - off:bo - off + bw],
                                    lhsT=wsb[:, ksl, m_sl],
                                    rhs=xs[:, ksl, bo:bo + bw],
                                    start=(j == 0),
                                    stop=(j == NKP - 1),
                                    perf_mode=DR,
                                )
                    if len(pending) > 0:
                        gate_back()
                    gate_front(m, kp, pp, tw, ts_sl)
            while pending:
                gate_back()

    nc.compile()
    return nc


def quantize_pack_x(x_b: np.ndarray, seq_len: int = S) -> np.ndarray:
    """x_b (seq, D) fp32 -> packed [128, nst, NK, tsp] fp8 (scaled by SX)."""
    tsp = min(1024, seq_len)
    nst = seq_len // tsp
    x8 = np.asarray(x_b * SX, dtype=NP_F8)
    return np.ascontiguousarray(
        x8.reshape(nst, tsp, NK, 128).transpose(3, 0, 2, 1)
    )


def quantize_pack_w(W: np.ndarray) -> np.ndarray:
    """W (H, D) fp32 -> packed [128, NK, H] fp8 (scaled by SW)."""
    W8 = np.asarray(W * SW, dtype=NP_F8)
    # w8[p, kt, m] = W[m, kt*128+p] * SW
    return np.ascontiguousarray(W8.T.reshape(NK, 128, H).transpose(1, 0, 2))


def make_in_maps(x, Wz, bz, Wh, bh, seq_len: int = S):
    wz8 = quantize_pack_w(np.asarray(Wz, np.float32))
    wh8 = quantize_pack_w(np.asarray(Wh, np.float32))
    bz = np.ascontiguousarray(bz, dtype=np.float32)
    bh = np.ascontiguousarray(bh, dtype=np.float32)
    return [
        {
            "xp8": quantize_pack_x(np.asarray(x[b], np.float32), seq_len),
            "wz8": wz8,
            "wh8": wh8,
            "bz": bz,
            "bh": bh,
        }
        for b in range(x.shape[0])
    ]


def kernel(x, Wz, bz, Wh, bh):
    x = np.ascontiguousarray(x, dtype=np.float32)
    key = "nc"
    if key not in _cache:
        _cache[key] = build_nc()
    nc = _cache[key]

    in_maps = make_in_maps(x, Wz, bz, Wh, bh)
    res = run_bass_kernel_spmd(nc, in_maps, list(range(N_CORES)))
    out = np.empty((B, S, H), np.float32)
    for b in range(N_CORES):
        out[b] = res.results[b]["hT"].astype(np.float32).T
    return out


# revision 15
# speedup vs baseline: 3.0672x; 1.1855x over previous
"""MinGRU Trainium2 kernel.

Problem: x (8, 4096, 1024) fp32; Wz, Wh (1024, 1024); bz, bh (1024,).
    k = x @ Wz.T + bz ; z = sigmoid(k)
    p = x @ Wh.T + bh ; g = where(p >= 0, p + 0.5, sigmoid(p))
    h_t = (1 - z_t) * h_{t-1} + z_t * g_t   (h_0 = 0.5)
The reference computes this recurrence with a log-space parallel scan; here it
is computed directly in linear space (mathematically identical), using the DVE
TensorTensorScanArith instruction along the free axis.

Sharding: data-parallel over batch, one batch element per NeuronCore (8 cores).

Per-core layout: everything lives transposed, H on partitions, S on the free
axis.  The two GEMMs run in fp8 e4m3 with perf_mode=DoubleRow (2 weights per
PE cell, K=256 per matmul -> half the matmul count of full-rate fp32).  Inputs
are quantized host-side with power-of-two scales (x*16, W*1024); the exact
descale 2^-14 is folded into the ScalarE activation `scale` argument.

Work is chunked in 1024-wide sequence units (PSUM tiles span 2 banks) to
amortize the per-instruction overheads (~352 cycles per ACT, ~200 per DVE op)
and halve the semaphore traffic.  Engine assignment balances the elementwise
work across ScalarE/GpSimd/DVE so the fp8 PE stream (~130 us) stays the
critical resource:
    ScalarE: z = sigmoid(kp), sp = sigmoid(pp), rp = relu(pp)   (bias fused)
    DVE:     g = min(sp,.5)+rp (STT is DVE-only on v3) ; h = scan(a, b)
    GpSimd:  a = 1 - z ; b = z*g
The scan is issued one (unit, m) slot late AND ahead of g in program order:
engine queues are strict FIFO, so the op at the DVE head must always have
ready inputs — scan(u-1)'s inputs are a slot old, while g(u) may still be
waiting on the Scalar.  z/sp/rp/a stay fp32 (bf16 tensor_scalar on GpSimd
measured 10x slower); g/b/h are stored bf16 (halves SBUF traffic on the
GpSimd/DVE shared port; scan reads (a fp32, b bf16) at the same serial rate
as fp32).  Work-pool buffer counts are sized so no engine stalls on tile
reuse (the z tile is read by GpSimd up to ~2 slots late).  Measured
end-to-end rel-err ~1.3e-2 against the fp32 reference, within 2e-2.
"""

import os
import sys

import numpy as np

for _p in ("/opt/trn_rl_repo", "/root/.axon_site/_ro/trn_rl_repo"):
    if os.path.isdir(_p) and _p not in sys.path:
        sys.path.insert(0, _p)

import ml_dtypes  # noqa: E402

import concourse.bass as bass  # noqa: E402
import concourse.mybir as mybir  # noqa: E402
import concourse.tile as tile  # noqa: E402
from concourse import bacc  # noqa: E402
from concourse.bass_utils import run_bass_kernel_spmd  # noqa: E402

F32 = mybir.dt.float32
F32R = mybir.dt.float32r
BF16 = mybir.dt.bfloat16
F8 = mybir.dt.float8e4  # TRN e4m3 (bias 8, max +-240) == ml_dtypes.float8_e4m3
NP_F8 = ml_dtypes.float8_e4m3
NP_BF16 = ml_dtypes.bfloat16
N_CORES = 8
B, S, D, H = 8, 4096, 1024, 1024
NK = D // 128  # 8 k-tiles of 128
NKP = NK // 2  # 4 DoubleRow k-pairs
NM = H // 128

# power-of-two quantization scales; descale folded into the activations
SX = 16.0
SW = 1024.0
DESCALE = 1.0 / (SX * SW)

_cache: dict = {}


def build_nc(seq_len: int = S, n_cores: int = N_CORES):
    """Build and compile the per-core Bass module (SPMD, identical program)."""
    tsp = min(1024, seq_len)  # strip width (2 PSUM banks of fp32 at 1024)
    nst = seq_len // tsp
    nc = bacc.Bacc(
        "TRN2", target_bir_lowering=False, debug=False, num_devices=n_cores
    )

    # x packed host-side as [p, strip, ktile, t] so one DMA fetches a strip
    xp_d = nc.dram_tensor("xp8", [128, nst, NK, tsp], F8, kind="ExternalInput")
    # weights packed as [p, ktile, m] (wz8[p, kt, m] = Wz[m, kt*128+p] * SW)
    wz_d = nc.dram_tensor("wz8", [128, NK, H], F8, kind="ExternalInput")
    wh_d = nc.dram_tensor("wh8", [128, NK, H], F8, kind="ExternalInput")
    bz_d = nc.dram_tensor("bz", [H], F32, kind="ExternalInput")
    bh_d = nc.dram_tensor("bh", [H], F32, kind="ExternalInput")
    hT_d = nc.dram_tensor("hT", [H, seq_len], BF16, kind="ExternalOutput")

    AF = mybir.ActivationFunctionType
    OP = mybir.AluOpType
    DR = mybir.MatmulPerfMode.DoubleRow

    with tile.TileContext(nc) as tc:
        with (
            tc.tile_pool(name="singles", bufs=1) as singles,
            tc.tile_pool(name="xs", bufs=3) as xpool,
            tc.tile_pool(name="work", bufs=3) as work,
            tc.tile_pool(name="hbuf", bufs=2) as hpool,
            tc.tile_pool(name="psum", bufs=2, space="PSUM") as psum,
        ):
            # PE warm-up: the HAM clock gate holds the PE at 1.2 GHz until it
            # has been busy ~3.4 us.  The PE sits idle anyway while the first
            # DMAs land, so burn that time on dummy matmuls over a zeroed
            # tile — the first real matmuls then run at 2.4 GHz.
            warm = singles.tile([128, 256], F32, tag="warm")
            nc.gpsimd.memset(warm[:], 0.0)
            wps = psum.tile([128, tsp], F32, tag="kp")
            for i in range(10):
                nc.tensor.matmul(
                    wps[:, :256], lhsT=warm[:, :128].bitcast(F32R),
                    rhs=warm[:].bitcast(F32R),
                    start=(i == 0), stop=(i == 9),
                )
            # Biases first: tiny but they gate every activation.
            bz_sb = singles.tile([128, NM], F32, tag="bz")
            nc.sync.dma_start(out=bz_sb, in_=bz_d.ap().rearrange("(m p) -> p m", p=128))
            bh_sb = singles.tile([128, NM], F32, tag="bh")
            nc.sync.dma_start(out=bh_sb, in_=bh_d.ap().rearrange("(m p) -> p m", p=128))
            # First strip of x + the m<4 half of the weights, interleaved per
            # k-pair so matmul (s=0, m=0, j=0) unblocks after ~3 transfers.
            xs0 = xpool.tile([128, NK, tsp], F8, tag="xs")
            wz_sb = singles.tile([128, NK, H], F8, tag="wz")
            wh_sb = singles.tile([128, NK, H], F8, tag="wh")
            for j in range(NKP):
                ksl = slice(2 * j, 2 * j + 2)
                nc.sync.dma_start(out=xs0[:, ksl, :], in_=xp_d.ap()[:, 0, ksl, :])
                nc.sync.dma_start(out=wz_sb[:, ksl, :H // 2],
                                  in_=wz_d.ap()[:, ksl, :H // 2])
                nc.sync.dma_start(out=wh_sb[:, ksl, :H // 2],
                                  in_=wh_d.ap()[:, ksl, :H // 2])
            for j in range(NKP):
                ksl = slice(2 * j, 2 * j + 2)
                nc.sync.dma_start(out=wz_sb[:, ksl, H // 2:],
                                  in_=wz_d.ap()[:, ksl, H // 2:])
                nc.sync.dma_start(out=wh_sb[:, ksl, H // 2:],
                                  in_=wh_d.ap()[:, ksl, H // 2:])
            # Sequence units: full strips of `tsp`, with the final strip split
            # in half so the end-of-kernel pipeline drain runs on narrower
            # tiles.
            units = [(s, 0, tsp) for s in range(nst - 1)]
            units += [(nst - 1, 0, tsp // 2), (nst - 1, tsp // 2, tsp // 2)]
            h_prev: list = [None] * NM
            pending: list = []

            def gate_front(m, kp, pp, tw, ts_sl):
                """ScalarE + DVE-g + GpSimd gate math for one (unit, m) slot."""
                z = work.tile([128, tsp], F32, tag="z", bufs=6)
                nc.scalar.activation(
                    out=z[:, :tw], in_=kp[:, :tw], func=AF.Sigmoid,
                    bias=bz_sb[:, m:m + 1], scale=DESCALE,
                )
                sp = work.tile([128, tsp], F32, tag="sp", bufs=4)
                nc.scalar.activation(
                    out=sp[:, :tw], in_=pp[:, :tw], func=AF.Sigmoid,
                    bias=bh_sb[:, m:m + 1], scale=DESCALE,
                )
                rp = work.tile([128, tsp], F32, tag="rp", bufs=4)
                nc.scalar.activation(
                    out=rp[:, :tw], in_=pp[:, :tw], func=AF.Relu,
                    bias=bh_sb[:, m:m + 1], scale=DESCALE,
                )
                # g = min(sigmoid(p+bh), 0.5) + relu(p+bh)
                g = work.tile([128, tsp], BF16, tag="g", bufs=4)
                nc.vector.scalar_tensor_tensor(
                    out=g[:, :tw], in0=sp[:, :tw], scalar=0.5, in1=rp[:, :tw],
                    op0=OP.min, op1=OP.add,
                )
                # a = 1 - z (GpSimd; kept light — heavy GpSimd SBUF traffic
                # contends with the DVE scan on their shared port)
                a = work.tile([128, tsp], F32, tag="a", bufs=4)
                nc.gpsimd.tensor_scalar(
                    out=a[:, :tw], in0=z[:, :tw], scalar1=-1.0, scalar2=1.0,
                    op0=OP.mult, op1=OP.add,
                )
                # b = z * g (DVE: z fp32 x g bf16 -> bf16 measured 0.8 ns/col)
                b = work.tile([128, tsp], BF16, tag="b", bufs=4)
                nc.vector.tensor_tensor(
                    out=b[:, :tw], in0=z[:, :tw], in1=g[:, :tw], op=OP.mult
                )
                pending.append((m, a, b, tw, ts_sl))

            def gate_back():
                """DVE scan + store, one slot behind gate_front."""
                m, a, b, tw, ts_sl = pending.pop(0)
                # h_t = a_t * h_{t-1} + b_t along the free axis
                h = hpool.tile([128, tsp], BF16, tag=f"h{m}")
                if h_prev[m] is None:
                    init = 0.5
                else:
                    pt, pw = h_prev[m]
                    init = pt[:, pw - 1:pw]
                nc.vector.tensor_tensor_scan(
                    out=h[:, :tw], data0=a[:, :tw], data1=b[:, :tw],
                    initial=init, op0=OP.mult, op1=OP.add,
                )
                h_prev[m] = (h, tw)
                nc.sync.dma_start(out=hT_d.ap()[m * 128:(m + 1) * 128, ts_sl],
                                  in_=h[:, :tw])

            for u, (sidx, off, tw) in enumerate(units):
                ts0 = sidx * tsp + off
                ts_sl = slice(ts0, ts0 + tw)
                if sidx == 0:
                    xs = xs0
                elif off == 0:
                    xs = xpool.tile([128, NK, tsp], F8, tag="xs")
                    nc.sync.dma_start(out=xs, in_=xp_d.ap()[:, sidx, :, :])
                # (tail sub-units reuse the strip tile loaded at off==0)
                blocks = [(off + i, min(512, tw - i)) for i in range(0, tw, 512)]
                for m in range(NM):
                    m_sl = slice(m * 128, (m + 1) * 128)
                    kp = psum.tile([128, tsp], F32, tag="kp")
                    pp = psum.tile([128, tsp], F32, tag="pp")
                    for wsb, out_ps in ((wz_sb, kp), (wh_sb, pp)):
                        for j in range(NKP):
                            ksl = slice(2 * j, 2 * j + 2)
                            for bo, bw in blocks:
                                nc.tensor.matmul(
                                    out_ps[:, bo - off:bo - off + bw],
                                    lhsT=wsb[:, ksl, m_sl],
                                    rhs=xs[:, ksl, bo:bo + bw],
                                    start=(j == 0),
                                    stop=(j == NKP - 1),
                                    perf_mode=DR,
                                )
                    if len(pending) > 0:
                        gate_back()
                    gate_front(m, kp, pp, tw, ts_sl)
            while pending:
                gate_back()

    nc.compile()
    return nc


def quantize_pack_x(x_b: np.ndarray, seq_len: int = S) -> np.ndarray:
    """x_b (seq, D) fp32 -> packed [128, nst, NK, tsp] fp8 (scaled by SX)."""
    tsp = min(1024, seq_len)
    nst = seq_len // tsp
    x8 = np.asarray(x_b * SX, dtype=NP_F8)
    return np.ascontiguousarray(
        x8.reshape(nst, tsp, NK, 128).transpose(3, 0, 2, 1)
    )


def quantize_pack_w(W: np.ndarray) -> np.ndarray:
    """W (H, D) fp32 -> packed [128, NK, H] fp8 (scaled by SW)."""
    W8 = np.asarray(W * SW, dtype=NP_F8)
    # w8[p, kt, m] = W[m, kt*128+p] * SW
    return np.ascontiguousarray(W8.T.reshape(NK, 128, H).transpose(1, 0, 2))


def make_in_maps(x, Wz, bz, Wh, bh, seq_len: int = S):
    wz8 = quantize_pack_w(np.asarray(Wz, np.float32))
    wh8 = quantize_pack_w(np.asarray(Wh, np.float32))
    bz = np.ascontiguousarray(bz, dtype=np.float32)
    bh = np.ascontiguousarray(bh, dtype=np.float32)
    return [
        {
            "xp8": quantize_pack_x(np.asarray(x[b], np.float32), seq_len),
            "wz8": wz8,
            "wh8": wh8,
            "bz": bz,
            "bh": bh,
        }
        for b in range(x.shape[0])
    ]


def kernel(x, Wz, bz, Wh, bh):
    x = np.ascontiguousarray(x, dtype=np.float32)
    key = "nc"
    if key not in _cache:
        _cache[key] = build_nc()
    nc = _cache[key]

    in_maps = make_in_maps(x, Wz, bz, Wh, bh)
    res = run_bass_kernel_spmd(nc, in_maps, list(range(N_CORES)))
    out = np.empty((B, S, H), np.float32)
    for b in range(N_CORES):
        out[b] = res.results[b]["hT"].astype(np.float32).T
    return out


# revision 18
# speedup vs baseline: 3.1690x; 1.0332x over previous
"""MinGRU Trainium2 kernel.

Problem: x (8, 4096, 1024) fp32; Wz, Wh (1024, 1024); bz, bh (1024,).
    k = x @ Wz.T + bz ; z = sigmoid(k)
    p = x @ Wh.T + bh ; g = where(p >= 0, p + 0.5, sigmoid(p))
    h_t = (1 - z_t) * h_{t-1} + z_t * g_t   (h_0 = 0.5)
The reference computes this recurrence with a log-space parallel scan; here it
is computed directly in linear space (mathematically identical), using the DVE
TensorTensorScanArith instruction along the free axis.

Sharding: data-parallel over batch, one batch element per NeuronCore (8 cores).

Per-core layout: everything lives transposed, H on partitions, S on the free
axis.  The two GEMMs run in fp8 e4m3 with perf_mode=DoubleRow (2 weights per
PE cell, K=256 per matmul -> half the matmul count of full-rate fp32).  Inputs
are quantized host-side with power-of-two scales (x*16, W*1024); the exact
descale 2^-14 is folded into the ScalarE activation `scale` argument.

Work is chunked in 1024-wide sequence units (PSUM tiles span 2 banks) to
amortize the per-instruction overheads (~352 cycles per ACT, ~200 per DVE op)
and halve the semaphore traffic.  Engine assignment balances the elementwise
work across ScalarE/GpSimd/DVE so the fp8 PE stream (~130 us) stays the
critical resource:
    ScalarE: z = sigmoid(kp), sp = sigmoid(pp), rp = relu(pp)   (bias fused)
    DVE:     g = min(sp,.5)+rp (STT is DVE-only on v3) ; h = scan(a, b)
    GpSimd:  a = 1 - z ; b = z*g
The scan is issued one (unit, m) slot late AND ahead of g in program order:
engine queues are strict FIFO, so the op at the DVE head must always have
ready inputs — scan(u-1)'s inputs are a slot old, while g(u) may still be
waiting on the Scalar.  ALL gate tensors and h are uniform fp32: measured on
hardware, dtype-uniform DVE ops run ~2x faster than mixed ones (tt 0.67 vs
1.1+ ns/col, stt 1.03 vs 1.8) and bf16 tensor_scalar on GpSimd is 10x
slower; the fp32 output DMA costs ~17 us on the non-critical Sync queue.
Work-pool buffer counts are sized so no engine stalls on tile reuse (the z
tile is read by GpSimd/DVE up to ~2 slots late).  Measured end-to-end
rel-err ~1.3e-2 against the fp32 reference, within 2e-2.
"""

import os
import sys

import numpy as np

for _p in ("/opt/trn_rl_repo", "/root/.axon_site/_ro/trn_rl_repo"):
    if os.path.isdir(_p) and _p not in sys.path:
        sys.path.insert(0, _p)

import ml_dtypes  # noqa: E402

import concourse.bass as bass  # noqa: E402
import concourse.mybir as mybir  # noqa: E402
import concourse.tile as tile  # noqa: E402
from concourse import bacc  # noqa: E402
from concourse.bass_utils import run_bass_kernel_spmd  # noqa: E402

F32 = mybir.dt.float32
F32R = mybir.dt.float32r
BF16 = mybir.dt.bfloat16
F8 = mybir.dt.float8e4  # TRN e4m3 (bias 8, max +-240) == ml_dtypes.float8_e4m3
NP_F8 = ml_dtypes.float8_e4m3
NP_BF16 = ml_dtypes.bfloat16
N_CORES = 8
B, S, D, H = 8, 4096, 1024, 1024
NK = D // 128  # 8 k-tiles of 128
NKP = NK // 2  # 4 DoubleRow k-pairs
NM = H // 128

# power-of-two quantization scales; descale folded into the activations
SX = 16.0
SW = 1024.0
DESCALE = 1.0 / (SX * SW)

_cache: dict = {}


def build_nc(seq_len: int = S, n_cores: int = N_CORES):
    """Build and compile the per-core Bass module (SPMD, identical program)."""
    tsp = min(1024, seq_len)  # strip width (2 PSUM banks of fp32 at 1024)
    nst = seq_len // tsp
    nc = bacc.Bacc(
        "TRN2", target_bir_lowering=False, debug=False, num_devices=n_cores
    )

    # x packed host-side as [p, strip, ktile, t] so one DMA fetches a strip
    xp_d = nc.dram_tensor("xp8", [128, nst, NK, tsp], F8, kind="ExternalInput")
    # weights packed as [p, ktile, m] (wz8[p, kt, m] = Wz[m, kt*128+p] * SW)
    wz_d = nc.dram_tensor("wz8", [128, NK, H], F8, kind="ExternalInput")
    wh_d = nc.dram_tensor("wh8", [128, NK, H], F8, kind="ExternalInput")
    bz_d = nc.dram_tensor("bz", [H], F32, kind="ExternalInput")
    bh_d = nc.dram_tensor("bh", [H], F32, kind="ExternalInput")
    hT_d = nc.dram_tensor("hT", [H, seq_len], F32, kind="ExternalOutput")

    AF = mybir.ActivationFunctionType
    OP = mybir.AluOpType
    DR = mybir.MatmulPerfMode.DoubleRow

    with tile.TileContext(nc) as tc:
        with (
            tc.tile_pool(name="singles", bufs=1) as singles,
            tc.tile_pool(name="xs", bufs=3) as xpool,
            tc.tile_pool(name="work", bufs=3) as work,
            tc.tile_pool(name="hbuf", bufs=2) as hpool,
            tc.tile_pool(name="psum", bufs=2, space="PSUM") as psum,
        ):
            # PE warm-up: the HAM clock gate holds the PE at 1.2 GHz until it
            # has been busy ~3.4 us.  The PE sits idle anyway while the first
            # DMAs land, so burn that time on dummy matmuls over a zeroed
            # tile — the first real matmuls then run at 2.4 GHz.
            warm = singles.tile([128, 256], F32, tag="warm")
            nc.gpsimd.memset(warm[:], 0.0)
            wps = psum.tile([128, tsp], F32, tag="kp")
            for i in range(10):
                nc.tensor.matmul(
                    wps[:, :256], lhsT=warm[:, :128].bitcast(F32R),
                    rhs=warm[:].bitcast(F32R),
                    start=(i == 0), stop=(i == 9),
                )
            # Biases first: tiny but they gate every activation.
            bz_sb = singles.tile([128, NM], F32, tag="bz")
            nc.sync.dma_start(out=bz_sb, in_=bz_d.ap().rearrange("(m p) -> p m", p=128))
            bh_sb = singles.tile([128, NM], F32, tag="bh")
            nc.sync.dma_start(out=bh_sb, in_=bh_d.ap().rearrange("(m p) -> p m", p=128))
            # First strip of x + the m<4 half of the weights, interleaved per
            # k-pair so matmul (s=0, m=0, j=0) unblocks after ~3 transfers.
            xs0 = xpool.tile([128, NK, tsp], F8, tag="xs")
            wz_sb = singles.tile([128, NK, H], F8, tag="wz")
            wh_sb = singles.tile([128, NK, H], F8, tag="wh")
            for j in range(NKP):
                ksl = slice(2 * j, 2 * j + 2)
                nc.sync.dma_start(out=xs0[:, ksl, :], in_=xp_d.ap()[:, 0, ksl, :])
                nc.sync.dma_start(out=wz_sb[:, ksl, :H // 2],
                                  in_=wz_d.ap()[:, ksl, :H // 2])
                nc.sync.dma_start(out=wh_sb[:, ksl, :H // 2],
                                  in_=wh_d.ap()[:, ksl, :H // 2])
            for j in range(NKP):
                ksl = slice(2 * j, 2 * j + 2)
                nc.sync.dma_start(out=wz_sb[:, ksl, H // 2:],
                                  in_=wz_d.ap()[:, ksl, H // 2:])
                nc.sync.dma_start(out=wh_sb[:, ksl, H // 2:],
                                  in_=wh_d.ap()[:, ksl, H // 2:])
            # Sequence units: full strips of `tsp`, with the final strip split
            # in half so the end-of-kernel pipeline drain runs on narrower
            # tiles.
            units = [(s, 0, tsp) for s in range(nst - 1)]
            units += [(nst - 1, 0, tsp // 2), (nst - 1, tsp // 2, tsp // 2)]
            h_prev: list = [None] * NM
            pending: list = []

            def gate_front(m, kp, pp, tw, ts_sl):
                """ScalarE + DVE-g + GpSimd gate math for one (unit, m) slot."""
                z = work.tile([128, tsp], F32, tag="z", bufs=4)
                nc.scalar.activation(
                    out=z[:, :tw], in_=kp[:, :tw], func=AF.Sigmoid,
                    bias=bz_sb[:, m:m + 1], scale=DESCALE,
                )
                sp = work.tile([128, tsp], F32, tag="sp", bufs=3)
                nc.scalar.activation(
                    out=sp[:, :tw], in_=pp[:, :tw], func=AF.Sigmoid,
                    bias=bh_sb[:, m:m + 1], scale=DESCALE,
                )
                rp = work.tile([128, tsp], F32, tag="rp", bufs=3)
                nc.scalar.activation(
                    out=rp[:, :tw], in_=pp[:, :tw], func=AF.Relu,
                    bias=bh_sb[:, m:m + 1], scale=DESCALE,
                )
                # g = min(sigmoid(p+bh), 0.5) + relu(p+bh)
                g = work.tile([128, tsp], F32, tag="g", bufs=3)
                nc.vector.scalar_tensor_tensor(
                    out=g[:, :tw], in0=sp[:, :tw], scalar=0.5, in1=rp[:, :tw],
                    op0=OP.min, op1=OP.add,
                )
                # a = 1 - z (DVE: GpSimd is left fully idle — it shares an
                # SBUF port with the DVE, and any concurrent GpSimd traffic
                # drops DVE ts/tt from their fast 2-port 2x mode to 1x)
                a = work.tile([128, tsp], F32, tag="a", bufs=4)
                nc.vector.tensor_scalar(
                    out=a[:, :tw], in0=z[:, :tw], scalar1=-1.0, scalar2=1.0,
                    op0=OP.mult, op1=OP.add,
                )
                # b = z * g (DVE; uniform fp32 runs in the fast 2x mode)
                b = work.tile([128, tsp], F32, tag="b", bufs=4)
                nc.vector.tensor_tensor(
                    out=b[:, :tw], in0=z[:, :tw], in1=g[:, :tw], op=OP.mult
                )
                pending.append((m, a, b, tw, ts_sl))

            def gate_back():
                """DVE scan + store, one slot behind gate_front."""
                m, a, b, tw, ts_sl = pending.pop(0)
                # h_t = a_t * h_{t-1} + b_t along the free axis
                h = hpool.tile([128, tsp], F32, tag=f"h{m}")
                if h_prev[m] is None:
                    init = 0.5
                else:
                    pt, pw = h_prev[m]
                    init = pt[:, pw - 1:pw]
                nc.vector.tensor_tensor_scan(
                    out=h[:, :tw], data0=a[:, :tw], data1=b[:, :tw],
                    initial=init, op0=OP.mult, op1=OP.add,
                )
                h_prev[m] = (h, tw)
                nc.sync.dma_start(out=hT_d.ap()[m * 128:(m + 1) * 128, ts_sl],
                                  in_=h[:, :tw])

            for u, (sidx, off, tw) in enumerate(units):
                ts0 = sidx * tsp + off
                ts_sl = slice(ts0, ts0 + tw)
                if sidx == 0:
                    xs = xs0
                elif off == 0:
                    xs = xpool.tile([128, NK, tsp], F8, tag="xs")
                    nc.sync.dma_start(out=xs, in_=xp_d.ap()[:, sidx, :, :])
                # (tail sub-units reuse the strip tile loaded at off==0)
                blocks = [(off + i, min(512, tw - i)) for i in range(0, tw, 512)]
                for m in range(NM):
                    m_sl = slice(m * 128, (m + 1) * 128)
                    kp = psum.tile([128, tsp], F32, tag="kp")
                    pp = psum.tile([128, tsp], F32, tag="pp")
                    for wsb, out_ps in ((wz_sb, kp), (wh_sb, pp)):
                        for j in range(NKP):
                            ksl = slice(2 * j, 2 * j + 2)
                            for bo, bw in blocks:
                                nc.tensor.matmul(
                                    out_ps[:, bo - off:bo - off + bw],
                                    lhsT=wsb[:, ksl, m_sl],
                                    rhs=xs[:, ksl, bo:bo + bw],
                                    start=(j == 0),
                                    stop=(j == NKP - 1),
                                    perf_mode=DR,
                                )
                    if len(pending) > 0:
                        gate_back()
                    gate_front(m, kp, pp, tw, ts_sl)
            while pending:
                gate_back()

    nc.compile()
    return nc


def quantize_pack_x(x_b: np.ndarray, seq_len: int = S) -> np.ndarray:
    """x_b (seq, D) fp32 -> packed [128, nst, NK, tsp] fp8 (scaled by SX)."""
    tsp = min(1024, seq_len)
    nst = seq_len // tsp
    x8 = np.asarray(x_b * SX, dtype=NP_F8)
    return np.ascontiguousarray(
        x8.reshape(nst, tsp, NK, 128).transpose(3, 0, 2, 1)
    )


def quantize_pack_w(W: np.ndarray) -> np.ndarray:
    """W (H, D) fp32 -> packed [128, NK, H] fp8 (scaled by SW)."""
    W8 = np.asarray(W * SW, dtype=NP_F8)
    # w8[p, kt, m] = W[m, kt*128+p] * SW
    return np.ascontiguousarray(W8.T.reshape(NK, 128, H).transpose(1, 0, 2))


def make_in_maps(x, Wz, bz, Wh, bh, seq_len: int = S):
    wz8 = quantize_pack_w(np.asarray(Wz, np.float32))
    wh8 = quantize_pack_w(np.asarray(Wh, np.float32))
    bz = np.ascontiguousarray(bz, dtype=np.float32)
    bh = np.ascontiguousarray(bh, dtype=np.float32)
    return [
        {
            "xp8": quantize_pack_x(np.asarray(x[b], np.float32), seq_len),
            "wz8": wz8,
            "wh8": wh8,
            "bz": bz,
            "bh": bh,
        }
        for b in range(x.shape[0])
    ]


def kernel(x, Wz, bz, Wh, bh):
    x = np.ascontiguousarray(x, dtype=np.float32)
    key = "nc"
    if key not in _cache:
        _cache[key] = build_nc()
    nc = _cache[key]

    in_maps = make_in_maps(x, Wz, bz, Wh, bh)
    res = run_bass_kernel_spmd(nc, in_maps, list(range(N_CORES)))
    out = np.empty((B, S, H), np.float32)
    for b in range(N_CORES):
        out[b] = res.results[b]["hT"].astype(np.float32).T
    return out


# revision 19
# speedup vs baseline: 3.2940x; 1.0395x over previous
"""MinGRU Trainium2 kernel.

Problem: x (8, 4096, 1024) fp32; Wz, Wh (1024, 1024); bz, bh (1024,).
    k = x @ Wz.T + bz ; z = sigmoid(k)
    p = x @ Wh.T + bh ; g = where(p >= 0, p + 0.5, sigmoid(p))
    h_t = (1 - z_t) * h_{t-1} + z_t * g_t   (h_0 = 0.5)
The reference computes this recurrence with a log-space parallel scan; here it
is computed directly in linear space (mathematically identical), using the DVE
TensorTensorScanArith instruction along the free axis.

Sharding: data-parallel over batch, one batch element per NeuronCore (8 cores).

Per-core layout: everything lives transposed, H on partitions, S on the free
axis.  The two GEMMs run in fp8 e4m3 with perf_mode=DoubleRow (2 weights per
PE cell, K=256 per matmul -> half the matmul count of full-rate fp32).  Inputs
are quantized host-side with power-of-two scales (x*16, W*1024); the exact
descale 2^-14 is folded into the ScalarE activation `scale` argument.

Work is chunked in 1024-wide sequence units (PSUM tiles span 2 banks) to
amortize the per-instruction overheads (~352 cycles per ACT, ~200 per DVE op)
and halve the semaphore traffic.  Engine assignment balances the elementwise
work across ScalarE/GpSimd/DVE so the fp8 PE stream (~130 us) stays the
critical resource:
    ScalarE: z = sigmoid(kp), sp = sigmoid(pp), rp = relu(pp)   (bias fused)
    DVE:     g = min(sp,.5)+rp (STT is DVE-only on v3) ; h = scan(a, b)
    GpSimd:  a = 1 - z ; b = z*g
The scan is issued one (unit, m) slot late AND ahead of g in program order:
engine queues are strict FIFO, so the op at the DVE head must always have
ready inputs — scan(u-1)'s inputs are a slot old, while g(u) may still be
waiting on the Scalar.  ALL gate tensors and h are uniform fp32: measured on
hardware, dtype-uniform DVE ops run ~2x faster than mixed ones (tt 0.67 vs
1.1+ ns/col, stt 1.03 vs 1.8) and bf16 tensor_scalar on GpSimd is 10x
slower; the fp32 output DMA costs ~17 us on the non-critical Sync queue.
Work-pool buffer counts are sized so no engine stalls on tile reuse (the z
tile is read by GpSimd/DVE up to ~2 slots late).  Measured end-to-end
rel-err ~1.3e-2 against the fp32 reference, within 2e-2.
"""

import os
import sys

import numpy as np

for _p in ("/opt/trn_rl_repo", "/root/.axon_site/_ro/trn_rl_repo"):
    if os.path.isdir(_p) and _p not in sys.path:
        sys.path.insert(0, _p)

import ml_dtypes  # noqa: E402

import concourse.bass as bass  # noqa: E402
import concourse.mybir as mybir  # noqa: E402
import concourse.tile as tile  # noqa: E402
from concourse import bacc  # noqa: E402
from concourse.bass_utils import run_bass_kernel_spmd  # noqa: E402

F32 = mybir.dt.float32
F32R = mybir.dt.float32r
BF16 = mybir.dt.bfloat16
F8 = mybir.dt.float8e4  # TRN e4m3 (bias 8, max +-240) == ml_dtypes.float8_e4m3
NP_F8 = ml_dtypes.float8_e4m3
NP_BF16 = ml_dtypes.bfloat16
N_CORES = 8
B, S, D, H = 8, 4096, 1024, 1024
NK = D // 128  # 8 k-tiles of 128
NKP = NK // 2  # 4 DoubleRow k-pairs
NM = H // 128

# power-of-two quantization scales; descale folded into the activations
SX = 16.0
SW = 1024.0
DESCALE = 1.0 / (SX * SW)

_cache: dict = {}


def build_nc(seq_len: int = S, n_cores: int = N_CORES):
    """Build and compile the per-core Bass module (SPMD, identical program)."""
    tsp = min(1024, seq_len)  # strip width (2 PSUM banks of fp32 at 1024)
    nst = seq_len // tsp
    nc = bacc.Bacc(
        "TRN2", target_bir_lowering=False, debug=False, num_devices=n_cores
    )

    # x packed host-side as [p, strip, ktile, t] so one DMA fetches a strip
    xp_d = nc.dram_tensor("xp8", [128, nst, NK, tsp], F8, kind="ExternalInput")
    # weights packed as [p, ktile, m] (wz8[p, kt, m] = Wz[m, kt*128+p] * SW)
    wz_d = nc.dram_tensor("wz8", [128, NK, H], F8, kind="ExternalInput")
    wh_d = nc.dram_tensor("wh8", [128, NK, H], F8, kind="ExternalInput")
    bz_d = nc.dram_tensor("bz", [H], F32, kind="ExternalInput")
    bzn_d = nc.dram_tensor("bzn", [H], F32, kind="ExternalInput")
    bh_d = nc.dram_tensor("bh", [H], F32, kind="ExternalInput")
    hT_d = nc.dram_tensor("hT", [H, seq_len], F32, kind="ExternalOutput")

    AF = mybir.ActivationFunctionType
    OP = mybir.AluOpType
    DR = mybir.MatmulPerfMode.DoubleRow

    with tile.TileContext(nc) as tc:
        with (
            tc.tile_pool(name="singles", bufs=1) as singles,
            tc.tile_pool(name="xs", bufs=3) as xpool,
            tc.tile_pool(name="work", bufs=3) as work,
            tc.tile_pool(name="hbuf", bufs=2) as hpool,
            tc.tile_pool(name="psum", bufs=2, space="PSUM") as psum,
        ):
            # PE warm-up: the HAM clock gate holds the PE at 1.2 GHz until it
            # has been busy ~3.4 us.  The PE sits idle anyway while the first
            # DMAs land, so burn that time on dummy matmuls over a zeroed
            # tile — the first real matmuls then run at 2.4 GHz.
            warm = singles.tile([128, 256], F32, tag="warm")
            nc.gpsimd.memset(warm[:], 0.0)
            wps = psum.tile([128, tsp], F32, tag="kp")
            for i in range(10):
                nc.tensor.matmul(
                    wps[:, :256], lhsT=warm[:, :128].bitcast(F32R),
                    rhs=warm[:].bitcast(F32R),
                    start=(i == 0), stop=(i == 9),
                )
            # Biases first: tiny but they gate every activation.
            bz_sb = singles.tile([128, NM], F32, tag="bz")
            nc.sync.dma_start(out=bz_sb, in_=bz_d.ap().rearrange("(m p) -> p m", p=128))
            bh_sb = singles.tile([128, NM], F32, tag="bh")
            nc.sync.dma_start(out=bh_sb, in_=bh_d.ap().rearrange("(m p) -> p m", p=128))
            bzn_sb = singles.tile([128, NM], F32, tag="bzn")
            nc.sync.dma_start(out=bzn_sb,
                              in_=bzn_d.ap().rearrange("(m p) -> p m", p=128))
            # First strip of x + the m<4 half of the weights, interleaved per
            # k-pair so matmul (s=0, m=0, j=0) unblocks after ~3 transfers.
            xs0 = xpool.tile([128, NK, tsp], F8, tag="xs")
            wz_sb = singles.tile([128, NK, H], F8, tag="wz")
            wh_sb = singles.tile([128, NK, H], F8, tag="wh")
            for j in range(NKP):
                ksl = slice(2 * j, 2 * j + 2)
                nc.sync.dma_start(out=xs0[:, ksl, :], in_=xp_d.ap()[:, 0, ksl, :])
                nc.sync.dma_start(out=wz_sb[:, ksl, :H // 2],
                                  in_=wz_d.ap()[:, ksl, :H // 2])
                nc.sync.dma_start(out=wh_sb[:, ksl, :H // 2],
                                  in_=wh_d.ap()[:, ksl, :H // 2])
            for j in range(NKP):
                ksl = slice(2 * j, 2 * j + 2)
                nc.sync.dma_start(out=wz_sb[:, ksl, H // 2:],
                                  in_=wz_d.ap()[:, ksl, H // 2:])
                nc.sync.dma_start(out=wh_sb[:, ksl, H // 2:],
                                  in_=wh_d.ap()[:, ksl, H // 2:])
            # Sequence units: full strips of `tsp`, with the final strip split
            # in half so the end-of-kernel pipeline drain runs on narrower
            # tiles.
            units = [(s, 0, tsp) for s in range(nst - 1)]
            units += [(nst - 1, 0, tsp // 2), (nst - 1, tsp // 2, tsp // 2)]
            h_prev: list = [None] * NM
            pending: list = []

            def gate_front(m, kp, pp, tw, ts_sl):
                """ScalarE + DVE-g + GpSimd gate math for one (unit, m) slot."""
                z = work.tile([128, tsp], F32, tag="z", bufs=4)
                nc.scalar.activation(
                    out=z[:, :tw], in_=kp[:, :tw], func=AF.Sigmoid,
                    bias=bz_sb[:, m:m + 1], scale=DESCALE,
                )
                # a = 1 - z = sigmoid(-(k+bz)): a 4th ACT on the Scalar
                # (which has slack) instead of a DVE tensor_scalar — the DVE
                # is the critical engine.  GpSimd stays fully idle: it shares
                # an SBUF port with the DVE, and any concurrent GpSimd
                # traffic drops DVE ops from their fast 2-port mode to 1x.
                a = work.tile([128, tsp], F32, tag="a", bufs=4)
                nc.scalar.activation(
                    out=a[:, :tw], in_=kp[:, :tw], func=AF.Sigmoid,
                    bias=bzn_sb[:, m:m + 1], scale=-DESCALE,
                )
                sp = work.tile([128, tsp], F32, tag="sp", bufs=3)
                nc.scalar.activation(
                    out=sp[:, :tw], in_=pp[:, :tw], func=AF.Sigmoid,
                    bias=bh_sb[:, m:m + 1], scale=DESCALE,
                )
                rp = work.tile([128, tsp], F32, tag="rp", bufs=3)
                nc.scalar.activation(
                    out=rp[:, :tw], in_=pp[:, :tw], func=AF.Relu,
                    bias=bh_sb[:, m:m + 1], scale=DESCALE,
                )
                # g = min(sigmoid(p+bh), 0.5) + relu(p+bh)
                g = work.tile([128, tsp], F32, tag="g", bufs=3)
                nc.vector.scalar_tensor_tensor(
                    out=g[:, :tw], in0=sp[:, :tw], scalar=0.5, in1=rp[:, :tw],
                    op0=OP.min, op1=OP.add,
                )
                # b = z * g (DVE; uniform fp32 runs in the fast 2x mode)
                b = work.tile([128, tsp], F32, tag="b", bufs=4)
                nc.vector.tensor_tensor(
                    out=b[:, :tw], in0=z[:, :tw], in1=g[:, :tw], op=OP.mult
                )
                pending.append((m, a, b, tw, ts_sl))

            def gate_back():
                """DVE scan + store, one slot behind gate_front."""
                m, a, b, tw, ts_sl = pending.pop(0)
                # h_t = a_t * h_{t-1} + b_t along the free axis
                h = hpool.tile([128, tsp], F32, tag=f"h{m}")
                if h_prev[m] is None:
                    init = 0.5
                else:
                    pt, pw = h_prev[m]
                    init = pt[:, pw - 1:pw]
                nc.vector.tensor_tensor_scan(
                    out=h[:, :tw], data0=a[:, :tw], data1=b[:, :tw],
                    initial=init, op0=OP.mult, op1=OP.add,
                )
                h_prev[m] = (h, tw)
                nc.sync.dma_start(out=hT_d.ap()[m * 128:(m + 1) * 128, ts_sl],
                                  in_=h[:, :tw])

            for u, (sidx, off, tw) in enumerate(units):
                ts0 = sidx * tsp + off
                ts_sl = slice(ts0, ts0 + tw)
                if sidx == 0:
                    xs = xs0
                elif off == 0:
                    xs = xpool.tile([128, NK, tsp], F8, tag="xs")
                    nc.sync.dma_start(out=xs, in_=xp_d.ap()[:, sidx, :, :])
                # (tail sub-units reuse the strip tile loaded at off==0)
                blocks = [(off + i, min(512, tw - i)) for i in range(0, tw, 512)]
                for m in range(NM):
                    m_sl = slice(m * 128, (m + 1) * 128)
                    kp = psum.tile([128, tsp], F32, tag="kp")
                    pp = psum.tile([128, tsp], F32, tag="pp")
                    for wsb, out_ps in ((wz_sb, kp), (wh_sb, pp)):
                        for j in range(NKP):
                            ksl = slice(2 * j, 2 * j + 2)
                            for bo, bw in blocks:
                                nc.tensor.matmul(
                                    out_ps[:, bo - off:bo - off + bw],
                                    lhsT=wsb[:, ksl, m_sl],
                                    rhs=xs[:, ksl, bo:bo + bw],
                                    start=(j == 0),
                                    stop=(j == NKP - 1),
                                    perf_mode=DR,
                                )
                    if len(pending) > 0:
                        gate_back()
                    gate_front(m, kp, pp, tw, ts_sl)
            while pending:
                gate_back()

    nc.compile()
    return nc


def quantize_pack_x(x_b: np.ndarray, seq_len: int = S) -> np.ndarray:
    """x_b (seq, D) fp32 -> packed [128, nst, NK, tsp] fp8 (scaled by SX)."""
    tsp = min(1024, seq_len)
    nst = seq_len // tsp
    x8 = np.asarray(x_b * SX, dtype=NP_F8)
    return np.ascontiguousarray(
        x8.reshape(nst, tsp, NK, 128).transpose(3, 0, 2, 1)
    )


def quantize_pack_w(W: np.ndarray) -> np.ndarray:
    """W (H, D) fp32 -> packed [128, NK, H] fp8 (scaled by SW)."""
    W8 = np.asarray(W * SW, dtype=NP_F8)
    # w8[p, kt, m] = W[m, kt*128+p] * SW
    return np.ascontiguousarray(W8.T.reshape(NK, 128, H).transpose(1, 0, 2))


def make_in_maps(x, Wz, bz, Wh, bh, seq_len: int = S):
    wz8 = quantize_pack_w(np.asarray(Wz, np.float32))
    wh8 = quantize_pack_w(np.asarray(Wh, np.float32))
    bz = np.ascontiguousarray(bz, dtype=np.float32)
    bh = np.ascontiguousarray(bh, dtype=np.float32)
    return [
        {
            "xp8": quantize_pack_x(np.asarray(x[b], np.float32), seq_len),
            "wz8": wz8,
            "wh8": wh8,
            "bz": bz,
            "bzn": np.ascontiguousarray(-bz),
            "bh": bh,
        }
        for b in range(x.shape[0])
    ]


def kernel(x, Wz, bz, Wh, bh):
    x = np.ascontiguousarray(x, dtype=np.float32)
    key = "nc"
    if key not in _cache:
        _cache[key] = build_nc()
    nc = _cache[key]

    in_maps = make_in_maps(x, Wz, bz, Wh, bh)
    res = run_bass_kernel_spmd(nc, in_maps, list(range(N_CORES)))
    out = np.empty((B, S, H), np.float32)
    for b in range(N_CORES):
        out[b] = res.results[b]["hT"].astype(np.float32).T
    return out


# revision 20
# speedup vs baseline: 3.4634x; 1.0514x over previous
"""MinGRU Trainium2 kernel.

Problem: x (8, 4096, 1024) fp32; Wz, Wh (1024, 1024); bz, bh (1024,).
    k = x @ Wz.T + bz ; z = sigmoid(k)
    p = x @ Wh.T + bh ; g = where(p >= 0, p + 0.5, sigmoid(p))
    h_t = (1 - z_t) * h_{t-1} + z_t * g_t   (h_0 = 0.5)
The reference computes this recurrence with a log-space parallel scan; here it
is computed directly in linear space (mathematically identical), using the DVE
TensorTensorScanArith instruction along the free axis.

Sharding: data-parallel over batch, one batch element per NeuronCore (8 cores).

Per-core layout: everything lives transposed, H on partitions, S on the free
axis.  The two GEMMs run in fp8 e4m3 with perf_mode=DoubleRow (2 weights per
PE cell, K=256 per matmul -> half the matmul count of full-rate fp32).  Inputs
are quantized host-side with power-of-two scales (x*16, W*1024); the exact
descale 2^-14 is folded into the ScalarE activation `scale` argument.

Work is chunked in 1024-wide sequence units (PSUM tiles span 2 banks) to
amortize the per-instruction overheads (~352 cycles per ACT, ~200 per DVE op)
and halve the semaphore traffic.  Engine assignment balances the elementwise
work across ScalarE/GpSimd/DVE so the fp8 PE stream (~130 us) stays the
critical resource:
    ScalarE: z = sigmoid(kp), sp = sigmoid(pp), rp = relu(pp)   (bias fused)
    DVE:     g = min(sp,.5)+rp (STT is DVE-only on v3) ; h = scan(a, b)
    GpSimd:  a = 1 - z ; b = z*g
The scan is issued one (unit, m) slot late AND ahead of g in program order:
engine queues are strict FIFO, so the op at the DVE head must always have
ready inputs — scan(u-1)'s inputs are a slot old, while g(u) may still be
waiting on the Scalar.  ALL gate tensors and h are uniform fp32: measured on
hardware, dtype-uniform DVE ops run ~2x faster than mixed ones (tt 0.67 vs
1.1+ ns/col, stt 1.03 vs 1.8) and bf16 tensor_scalar on GpSimd is 10x
slower; the fp32 output DMA costs ~17 us on the non-critical Sync queue.
Work-pool buffer counts are sized so no engine stalls on tile reuse (the z
tile is read by GpSimd/DVE up to ~2 slots late).  Measured end-to-end
rel-err ~1.3e-2 against the fp32 reference, within 2e-2.
"""

import os
import sys

import numpy as np

for _p in ("/opt/trn_rl_repo", "/root/.axon_site/_ro/trn_rl_repo"):
    if os.path.isdir(_p) and _p not in sys.path:
        sys.path.insert(0, _p)

import ml_dtypes  # noqa: E402

import concourse.bass as bass  # noqa: E402
import concourse.mybir as mybir  # noqa: E402
import concourse.tile as tile  # noqa: E402
from concourse import bacc  # noqa: E402
from concourse.bass_utils import run_bass_kernel_spmd  # noqa: E402

F32 = mybir.dt.float32
F32R = mybir.dt.float32r
BF16 = mybir.dt.bfloat16
F8 = mybir.dt.float8e4  # TRN e4m3 (bias 8, max +-240) == ml_dtypes.float8_e4m3
NP_F8 = ml_dtypes.float8_e4m3
NP_BF16 = ml_dtypes.bfloat16
N_CORES = 8
B, S, D, H = 8, 4096, 1024, 1024
NK = D // 128  # 8 k-tiles of 128
NKP = NK // 2  # 4 DoubleRow k-pairs
NM = H // 128

# power-of-two quantization scales; descale folded into the activations
SX = 16.0
SW = 1024.0
DESCALE = 1.0 / (SX * SW)

_cache: dict = {}


def build_nc(seq_len: int = S, n_cores: int = N_CORES):
    """Build and compile the per-core Bass module (SPMD, identical program)."""
    tsp = min(1024, seq_len)  # strip width (2 PSUM banks of fp32 at 1024)
    nst = seq_len // tsp
    nc = bacc.Bacc(
        "TRN2", target_bir_lowering=False, debug=False, num_devices=n_cores
    )

    # x packed host-side as [p, strip, ktile, t] so one DMA fetches a strip
    xp_d = nc.dram_tensor("xp8", [128, nst, NK, tsp], F8, kind="ExternalInput")
    # weights packed as [p, ktile, m] (wz8[p, kt, m] = Wz[m, kt*128+p] * SW)
    wz_d = nc.dram_tensor("wz8", [128, NK, H], F8, kind="ExternalInput")
    wh_d = nc.dram_tensor("wh8", [128, NK, H], F8, kind="ExternalInput")
    bz_d = nc.dram_tensor("bz", [H], F32, kind="ExternalInput")
    bzn_d = nc.dram_tensor("bzn", [H], F32, kind="ExternalInput")
    bh_d = nc.dram_tensor("bh", [H], F32, kind="ExternalInput")
    hT_d = nc.dram_tensor("hT", [H, seq_len], F32, kind="ExternalOutput")

    AF = mybir.ActivationFunctionType
    OP = mybir.AluOpType
    DR = mybir.MatmulPerfMode.DoubleRow

    with tile.TileContext(nc) as tc:
        with (
            tc.tile_pool(name="singles", bufs=1) as singles,
            tc.tile_pool(name="xs", bufs=3) as xpool,
            tc.tile_pool(name="work", bufs=3) as work,
            tc.tile_pool(name="hbuf", bufs=2) as hpool,
            tc.tile_pool(name="psum", bufs=2, space="PSUM") as psum,
        ):
            # PE warm-up: the HAM clock gate holds the PE at 1.2 GHz until it
            # has been busy ~3.4 us.  The PE sits idle anyway while the first
            # DMAs land, so burn that time on dummy matmuls over a zeroed
            # tile — the first real matmuls then run at 2.4 GHz.
            warm = singles.tile([128, 256], F32, tag="warm")
            nc.gpsimd.memset(warm[:], 0.0)
            wps = psum.tile([128, tsp], F32, tag="kp")
            for i in range(10):
                nc.tensor.matmul(
                    wps[:, :256], lhsT=warm[:, :128].bitcast(F32R),
                    rhs=warm[:].bitcast(F32R),
                    start=(i == 0), stop=(i == 9),
                )
            # Biases first: tiny but they gate every activation.
            bz_sb = singles.tile([128, NM], F32, tag="bz")
            nc.sync.dma_start(out=bz_sb, in_=bz_d.ap().rearrange("(m p) -> p m", p=128))
            bh_sb = singles.tile([128, NM], F32, tag="bh")
            nc.sync.dma_start(out=bh_sb, in_=bh_d.ap().rearrange("(m p) -> p m", p=128))
            bzn_sb = singles.tile([128, NM], F32, tag="bzn")
            nc.sync.dma_start(out=bzn_sb,
                              in_=bzn_d.ap().rearrange("(m p) -> p m", p=128))
            # First strip of x + the m<4 half of the weights, interleaved per
            # k-pair so matmul (s=0, m=0, j=0) unblocks after ~3 transfers.
            xs0 = xpool.tile([128, NK, tsp], F8, tag="xs")
            wz_sb = singles.tile([128, NK, H], F8, tag="wz")
            wh_sb = singles.tile([128, NK, H], F8, tag="wh")
            for j in range(NKP):
                ksl = slice(2 * j, 2 * j + 2)
                nc.sync.dma_start(out=xs0[:, ksl, :], in_=xp_d.ap()[:, 0, ksl, :])
                nc.sync.dma_start(out=wz_sb[:, ksl, :H // 2],
                                  in_=wz_d.ap()[:, ksl, :H // 2])
                nc.sync.dma_start(out=wh_sb[:, ksl, :H // 2],
                                  in_=wh_d.ap()[:, ksl, :H // 2])
            for j in range(NKP):
                ksl = slice(2 * j, 2 * j + 2)
                nc.sync.dma_start(out=wz_sb[:, ksl, H // 2:],
                                  in_=wz_d.ap()[:, ksl, H // 2:])
                nc.sync.dma_start(out=wh_sb[:, ksl, H // 2:],
                                  in_=wh_d.ap()[:, ksl, H // 2:])
            # Sequence units: full strips of `tsp`, with the final strip split
            # in half so the end-of-kernel pipeline drain runs on narrower
            # tiles.
            units = [(s, 0, tsp) for s in range(nst - 1)]
            units += [(nst - 1, 0, tsp // 2), (nst - 1, tsp // 2, tsp // 2)]
            h_prev: list = [None] * NM
            pending: list = []

            def gate_front(m, kp, pp, tw, ts_sl):
                """ScalarE + DVE-g + GpSimd gate math for one (unit, m) slot."""
                # a = 1 - z = sigmoid(-(k+bz)): a 4th ACT on the Scalar
                # (which has slack) instead of a DVE tensor_scalar — the DVE
                # is the critical engine.  GpSimd stays fully idle: it shares
                # an SBUF port with the DVE, and any concurrent GpSimd
                # traffic drops DVE ops from their fast 2-port mode to 1x.
                a = work.tile([128, tsp], F32, tag="a", bufs=4)
                nc.scalar.activation(
                    out=a[:, :tw], in_=kp[:, :tw], func=AF.Sigmoid,
                    bias=bzn_sb[:, m:m + 1], scale=-DESCALE,
                )
                sp = work.tile([128, tsp], F32, tag="sp", bufs=3)
                nc.scalar.activation(
                    out=sp[:, :tw], in_=pp[:, :tw], func=AF.Sigmoid,
                    bias=bh_sb[:, m:m + 1], scale=DESCALE,
                )
                rp = work.tile([128, tsp], F32, tag="rp", bufs=3)
                nc.scalar.activation(
                    out=rp[:, :tw], in_=pp[:, :tw], func=AF.Relu,
                    bias=bh_sb[:, m:m + 1], scale=DESCALE,
                )
                # g = min(sigmoid(p+bh), 0.5) + relu(p+bh)
                g = work.tile([128, tsp], F32, tag="g", bufs=3)
                nc.vector.scalar_tensor_tensor(
                    out=g[:, :tw], in0=sp[:, :tw], scalar=0.5, in1=rp[:, :tw],
                    op0=OP.min, op1=OP.add,
                )
                # b_neg = (a-1)*g = -z*g in one DVE stt — the z activation
                # is gone entirely (z = 1-a); the scan compensates with
                # op1=subtract: h = a*h - b_neg = a*h + z*g.
                b = work.tile([128, tsp], F32, tag="b", bufs=4)
                nc.vector.scalar_tensor_tensor(
                    out=b[:, :tw], in0=a[:, :tw], scalar=1.0, in1=g[:, :tw],
                    op0=OP.subtract, op1=OP.mult,
                )
                pending.append((m, a, b, tw, ts_sl))

            def gate_back():
                """DVE scan + store, one slot behind gate_front."""
                m, a, b, tw, ts_sl = pending.pop(0)
                # h_t = a_t * h_{t-1} + b_t along the free axis
                h = hpool.tile([128, tsp], F32, tag=f"h{m}")
                if h_prev[m] is None:
                    init = 0.5
                else:
                    pt, pw = h_prev[m]
                    init = pt[:, pw - 1:pw]
                nc.vector.tensor_tensor_scan(
                    out=h[:, :tw], data0=a[:, :tw], data1=b[:, :tw],
                    initial=init, op0=OP.mult, op1=OP.subtract,
                )
                h_prev[m] = (h, tw)
                nc.sync.dma_start(out=hT_d.ap()[m * 128:(m + 1) * 128, ts_sl],
                                  in_=h[:, :tw])

            for u, (sidx, off, tw) in enumerate(units):
                ts0 = sidx * tsp + off
                ts_sl = slice(ts0, ts0 + tw)
                if sidx == 0:
                    xs = xs0
                elif off == 0:
                    xs = xpool.tile([128, NK, tsp], F8, tag="xs")
                    nc.sync.dma_start(out=xs, in_=xp_d.ap()[:, sidx, :, :])
                # (tail sub-units reuse the strip tile loaded at off==0)
                blocks = [(off + i, min(512, tw - i)) for i in range(0, tw, 512)]
                for m in range(NM):
                    m_sl = slice(m * 128, (m + 1) * 128)
                    kp = psum.tile([128, tsp], F32, tag="kp")
                    pp = psum.tile([128, tsp], F32, tag="pp")
                    for wsb, out_ps in ((wz_sb, kp), (wh_sb, pp)):
                        for j in range(NKP):
                            ksl = slice(2 * j, 2 * j + 2)
                            for bo, bw in blocks:
                                nc.tensor.matmul(
                                    out_ps[:, bo - off:bo - off + bw],
                                    lhsT=wsb[:, ksl, m_sl],
                                    rhs=xs[:, ksl, bo:bo + bw],
                                    start=(j == 0),
                                    stop=(j == NKP - 1),
                                    perf_mode=DR,
                                )
                    if len(pending) > 0:
                        gate_back()
                    gate_front(m, kp, pp, tw, ts_sl)
            while pending:
                gate_back()

    nc.compile()
    return nc


def quantize_pack_x(x_b: np.ndarray, seq_len: int = S) -> np.ndarray:
    """x_b (seq, D) fp32 -> packed [128, nst, NK, tsp] fp8 (scaled by SX)."""
    tsp = min(1024, seq_len)
    nst = seq_len // tsp
    x8 = np.asarray(x_b * SX, dtype=NP_F8)
    return np.ascontiguousarray(
        x8.reshape(nst, tsp, NK, 128).transpose(3, 0, 2, 1)
    )


def quantize_pack_w(W: np.ndarray) -> np.ndarray:
    """W (H, D) fp32 -> packed [128, NK, H] fp8 (scaled by SW)."""
    W8 = np.asarray(W * SW, dtype=NP_F8)
    # w8[p, kt, m] = W[m, kt*128+p] * SW
    return np.ascontiguousarray(W8.T.reshape(NK, 128, H).transpose(1, 0, 2))


def make_in_maps(x, Wz, bz, Wh, bh, seq_len: int = S):
    wz8 = quantize_pack_w(np.asarray(Wz, np.float32))
    wh8 = quantize_pack_w(np.asarray(Wh, np.float32))
    bz = np.ascontiguousarray(bz, dtype=np.float32)
    bh = np.ascontiguousarray(bh, dtype=np.float32)
    return [
        {
            "xp8": quantize_pack_x(np.asarray(x[b], np.float32), seq_len),
            "wz8": wz8,
            "wh8": wh8,
            "bz": bz,
            "bzn": np.ascontiguousarray(-bz),
            "bh": bh,
        }
        for b in range(x.shape[0])
    ]


def kernel(x, Wz, bz, Wh, bh):
    x = np.ascontiguousarray(x, dtype=np.float32)
    key = "nc"
    if key not in _cache:
        _cache[key] = build_nc()
    nc = _cache[key]

    in_maps = make_in_maps(x, Wz, bz, Wh, bh)
    res = run_bass_kernel_spmd(nc, in_maps, list(range(N_CORES)))
    out = np.empty((B, S, H), np.float32)
    for b in range(N_CORES):
        out[b] = res.results[b]["hT"].astype(np.float32).T
    return out
